# revision 25
# baseline (speedup 1.0000x reference)
"""AttentionBlock kernel for 8 Trainium2 NeuronCores.

Problem: x(8,512,32,32) -> GroupNorm(32) -> qkv 1x1 conv -> 8-head attention
         over T=1024 tokens -> proj 1x1 conv -> residual.

Sharding: pure data parallel - one batch element per core, no collectives.

Per-core dataflow (c=512 channels on partitions, T=1024 tokens on free dim):
  1. GroupNorm stats: bn_stats per channel; tiny PE matmuls (group-selector
     matrices) aggregate across the 16 channels of each group and broadcast
     group stats back to channels; xn = x*A + B via one tensor_scalar per tile.
  2. q,k projection (fp32r) with head-pair-permuted weights; outputs cast to
     bf16: q/k of heads (2p,2p+1) stacked on partitions 0-63/64-127 of one
     (128,1024) tile each, so the K=64 score matmuls of a pair land in
     disjoint PE row groups.
  3. v is produced TRANSPOSED directly by the matmul vT = xn^T @ wvT
     (lhsT = xn), avoiding any explicit transpose for the second attention
     matmul; cast to bf16 with an all-ones column appended (M=65) so the WV
     matmul also emits the softmax denominator D[t] as output row 64.
  4. scores^T(s,t) = k^T q per head in bf16 (hw runs K<128 fp32r matmuls at
     half rate; bf16 runs at full rate and the ~4e-3 rounding is well within
     tolerance).
  5. exp on the scalar engine reading PSUM, writing bf16 (scale=0.125 folds
     the attention scaling exactly); WV accumulates hu (and D at row 64) in
     fp32 PSUM. 1/D via reciprocal_approx_fast after a K=1 ones-matmul
     broadcasts D to all partitions.
  6. proj (fp32r) + residual into the x tiles, DMA out.

Schedule: WV runs 3 pipeline steps behind scores/exp so the scalar engine
never starves; the previous pair's division chain is emitted inside the next
pair (keeps the in-order PE queue from stalling on it); qk chunks 2..7 and
all vT matmuls are spread across pair 0's pipeline steps.
"""

import sys

for _p in ("/opt/trn_rl_repo",):
    if _p not in sys.path:
        sys.path.insert(0, _p)

import numpy as np

import concourse.bass as bass
import concourse.tile as tile
from concourse import mybir
from concourse.bass_utils import run_bass_kernel_spmd
from concourse.vector_clock import ScopedClock, VectorClock


def _patched_drain_and_barrier(self, tick_clock, wait_clock):
    # This container's walrus rejects instructions carrying more than one
    # sync wait. Split the final drain's global-clock waits across NOPs.
    g = tick_clock.global_clock
    n = len(g)
    for lo in range(0, n, 4):
        vec = [g[p] if lo <= p < lo + 4 else 0 for p in range(n)]
        if not any(vec):
            continue
        nop_inst = self.nc.sync.nop()
        wait_clock.add_sem_waits(nop_inst.ins, ScopedClock({None: VectorClock(vec)}))
    self.nc.sync.drain()
    self.nc.all_engine_barrier()
    assert self.sems is not None
    popped = self.nc._tile_sem_poison_stack.pop()
    assert popped is self._sem_poison
    self.nc.clear_and_free_semaphores(list(self.sems.allocated().values()))
    self.nc.all_engine_barrier()


tile.TileContext._drain_and_barrier = _patched_drain_and_barrier


def _split_multi_waits(nc):
    # This walrus build accepts at most one sync wait per instruction. Hoist
    # surplus waits onto same-engine NOPs placed immediately before.
    n = 0
    for fn in nc.m.functions:
        for blk in fn.blocks:
            out = []
            for inst in blk.instructions:
                si = inst.sync_info
                if si is not None and si.on_wait and len(si.on_wait) > 1:
                    waits = list(si.on_wait)
                    for w in waits[:-1]:
                        nop = mybir.InstNoOp(name=f"{inst.name}_w{n}", ins=[], outs=[])
                        n += 1
                        nop.engine = inst.engine
                        nop.sync_info = mybir.SyncInfo(on_wait=[w], on_update=[])
                        out.append(nop)
                    si.on_wait = [waits[-1]]
                out.append(inst)
            blk.instructions = out


F32 = mybir.dt.float32
F32R = mybir.dt.float32r
BF16 = mybir.dt.bfloat16

C = 512
T = 1024
NH = 8          # heads
CH = C // NH    # 64 channels per head
NG = 32         # groups
GS = C // NG    # 16 channels per group
EPS = 1e-5
NCORES = 8
CT = C // 128   # 4 channel tiles
ST = T // 128   # 8 s tiles
TC = T // 512   # 2 t chunks
DELAY = 3       # wv runs this many pipeline steps behind scores/exp


def build_program(use_bqk: bool, use_bproj: bool) -> bass.Bass:
    nc = bass.Bass()

    x_d = nc.dram_tensor("x", [C, T], F32R, kind="ExternalInput")
    wqk_d = nc.dram_tensor("wqk", [128, CT, 2 * C], F32R, kind="ExternalInput")
    wv_d = nc.dram_tensor("wv", [128, CT, C], F32R, kind="ExternalInput")
    wproj_d = nc.dram_tensor("wproj", [128, CT, C], F32R, kind="ExternalInput")
    gsel_d = nc.dram_tensor("gsel", [128, CT, NG], F32R, kind="ExternalInput")
    gselt_d = nc.dram_tensor("gselt", [NG, C], F32R, kind="ExternalInput")
    # combined small params: [bqk(8) | bproj(4) | gamma(4) | beta(4)]
    bias_d = nc.dram_tensor("biases", [128, 20], F32, kind="ExternalInput")
    out_d = nc.dram_tensor("out", [C, T], F32R, kind="ExternalOutput")

    with tile.TileContext(nc) as tc:
        with (
            tc.tile_pool(name="persist", bufs=1) as P,
            tc.tile_pool(name="work", bufs=2) as W,
            tc.tile_pool(name="ps", bufs=2, space="PSUM") as PS,
            tc.tile_pool(name="dr", bufs=2, space="DRAM") as DR,
        ):
            _body(nc, tc, P, W, PS, DR, locals(), use_bqk, use_bproj)
    return nc


def _body(nc, tc, P, W, PS, DR, d, use_bqk, use_bproj):
    x_d, wqk_d, wv_d, wproj_d = d["x_d"], d["wqk_d"], d["wv_d"], d["wproj_d"]
    gsel_d, gselt_d, bias_d, out_d = d["gsel_d"], d["gselt_d"], d["bias_d"], d["out_d"]

    # ---- persistent SBUF tiles ----
    x_t = [P.tile([128, T], F32R, name=f"x{i}", tag=f"x{i}") for i in range(CT)]
    xn_t = [P.tile([128, T], F32R, name=f"xn{i}", tag=f"xn{i}") for i in range(CT)]
    h_t = [P.tile([128, T], F32R, name=f"h{i}", tag=f"h{i}") for i in range(CT)]
    q_t = [P.tile([128, T], BF16, name=f"q{p}", tag=f"q{p}") for p in range(4)]
    k_t = [P.tile([128, T], BF16, name=f"k{p}", tag=f"k{p}") for p in range(4)]
    vaug_t = [P.tile([128, NH, CH + 1], BF16, name=f"va{s}", tag=f"va{s}") for s in range(ST)]
    wqk_t = P.tile([128, CT, 2 * C], F32R, name="wqk_t")
    wv_t = P.tile([128, CT, C], F32R, name="wv_t")
    wp_t = P.tile([128, CT, C], F32R, name="wp_t")
    gsel_t = P.tile([128, CT, NG], F32R, name="gsel_t")
    gselt_t = P.tile([NG, C], F32R, name="gselt_t")
    ab_t = [P.tile([128, 2], F32, name=f"ab{i}", tag=f"ab{i}") for i in range(CT)]
    bias_sb = P.tile([128, 20], F32, name="bias_sb")
    bqk_sb = bias_sb[:, 0:8]
    bproj_sb = bias_sb[:, 8:12]
    gamma_sb = bias_sb[:, 12:16]
    beta_sb = bias_sb[:, 16:20]
    eps_sb = P.tile([NG, 1], F32, name="eps_sb")
    mvr_sb = P.tile([NG, 2], F32R, name="mvr_sb")

    # ---- input DMAs: x on sync, weights on the scalar HWDGE queue ----
    for i in range(CT):
        for half in range(2):
            fs = slice(half * 512, half * 512 + 512)
            nc.sync.dma_start(out=x_t[i][:, fs], in_=x_d[i * 128:(i + 1) * 128, fs])
    # weights go through the SWDGE queues (gpsimd): separate semaphore pool,
    # so a big weight transfer never blocks an x tile's HWDGE slot. wqk is
    # split so pair 0's q/k columns land first (gates the first scores).
    nc.gpsimd.dma_start(out=gsel_t, in_=gsel_d[:, :, :])
    nc.gpsimd.dma_start(out=gselt_t, in_=gselt_d[:, :])
    nc.gpsimd.dma_start(out=bias_sb, in_=bias_d[:, :])
    nc.gpsimd.dma_start(out=wqk_t[:, :, 0:256], in_=wqk_d[:, :, 0:256])
    nc.gpsimd.dma_start(out=wqk_t[:, :, 256:2 * C], in_=wqk_d[:, :, 256:2 * C])
    nc.gpsimd.dma_start(out=wv_t, in_=wv_d[:, :, :])
    nc.gpsimd.dma_start(out=wp_t, in_=wproj_d[:, :, :])
    nc.vector.memset(eps_sb, EPS)

    # ---- phase 1: group norm statistics ----
    # per-channel sum via DVE tensor_scalar accumulate (dumps into xn, which
    # is fully overwritten later) and sum(x^2) via ACT Square accumulate
    # (dumps into h_t, dead until attention); both run while x halves land
    mv32_ps = PS.tile([NG, 2], F32, name="mv32_ps", tag="sc")
    for i in range(CT):
        st2 = W.tile([128, 2], F32R, name="st2", tag="st2", bufs=4)
        with nc.allow_low_precision(reason="f32r accum tags; same 4B payload"):
            nc.vector.tensor_scalar(out=xn_t[i], in0=x_t[i],
                                    scalar1=1.0, scalar2=0.0,
                                    op0=mybir.AluOpType.mult,
                                    op1=mybir.AluOpType.add,
                                    accum_out=st2[:, 0:1])
            nc.scalar.activation(out=h_t[i], in_=x_t[i],
                                 func=mybir.ActivationFunctionType.Square,
                                 accum_out=st2[:, 1:2])
        # group aggregation: (1/(16*1024)) * indicator^T @ [sum, sumsq]
        nc.tensor.matmul(out=mv32_ps, lhsT=gsel_t[:, i, :], rhs=st2,
                         start=(i == 0), stop=(i == CT - 1))
    mv32_sb = W.tile([NG, 2], F32, name="mv32_sb", tag="mv32", bufs=1)
    sdtmp = W.tile([NG, 2], F32, name="sdtmp", tag="sdtmp", bufs=1)
    nc.vector.tensor_copy(out=mv32_sb, in_=mv32_ps)
    # var_g = E[x^2]_g - mean_g^2 ; rstd = 1/sqrt(var+eps)
    nc.vector.tensor_mul(out=sdtmp[:, 0:1], in0=mv32_sb[:, 0:1], in1=mv32_sb[:, 0:1])
    nc.vector.tensor_sub(out=sdtmp[:, 1:2], in0=mv32_sb[:, 1:2], in1=sdtmp[:, 0:1])
    nc.scalar.activation(out=sdtmp[:, 1:2], in_=sdtmp[:, 1:2],
                         func=mybir.ActivationFunctionType.Sqrt,
                         bias=eps_sb, scale=1.0)
    with nc.allow_low_precision(reason="f32r tag on rstd; same 4-byte payload"):
        nc.vector.reciprocal(out=mvr_sb[:, 1:2], in_=sdtmp[:, 1:2])
    nc.vector.tensor_copy(out=mvr_sb[:, 0:1], in_=mv32_sb[:, 0:1])
    # broadcast group stats back to channels, per-channel A/B, xn = x*A + B
    for i in range(CT):
        mr_ps = PS.tile([128, 2], F32, name="mr_ps", tag="sc")
        nc.tensor.matmul(out=mr_ps, lhsT=gselt_t[:, i * 128:(i + 1) * 128],
                         rhs=mvr_sb, start=True, stop=True)
        abm = W.tile([128, 1], F32, name="abm", tag="abm", bufs=4)
        nc.vector.tensor_mul(out=ab_t[i][:, 0:1], in0=mr_ps[:, 1:2], in1=gamma_sb[:, i:i + 1])
        nc.vector.tensor_mul(out=abm, in0=mr_ps[:, 0:1], in1=ab_t[i][:, 0:1])
        nc.vector.tensor_sub(out=ab_t[i][:, 1:2], in0=beta_sb[:, i:i + 1], in1=abm)
        nc.vector.tensor_scalar(out=xn_t[i], in0=x_t[i],
                                scalar1=ab_t[i][:, 0:1], scalar2=ab_t[i][:, 1:2],
                                op0=mybir.AluOpType.mult, op1=mybir.AluOpType.add)

    # ---- deferred PE work: 4-matmul units spread across pairs 0..2 ----
    def qk_unit(oc, t2):
        p, is_k = oc // 2, oc % 2
        fs = slice(t2 * 512, t2 * 512 + 512)
        ps = PS.tile([128, 512], F32, name="qkps", tag="sc")
        for kk in range(CT):
            nc.tensor.matmul(out=ps,
                             lhsT=wqk_t[:, kk, oc * 128:(oc + 1) * 128],
                             rhs=xn_t[kk][:, fs],
                             start=(kk == 0), stop=(kk == CT - 1))
        dst = (k_t[p] if is_k else q_t[p])[:, fs]
        if use_bqk:
            nc.vector.tensor_scalar(out=dst, in0=ps, scalar1=bqk_sb[:, oc:oc + 1],
                                    scalar2=None, op0=mybir.AluOpType.add)
        else:
            nc.vector.tensor_copy(out=dst, in_=ps)

    def vt_unit(s):
        # whole-tile memset (strided ones-column memset fails this walrus);
        # the copy below overwrites cols 0..63 per head, col 64 stays 1.0
        nc.vector.memset(vaug_t[s], 1.0)
        ps = PS.tile([128, C], F32, name="vtps", tag="sc")
        for kk in range(CT):
            nc.tensor.matmul(out=ps,
                             lhsT=xn_t[kk][:, s * 128:(s + 1) * 128],
                             rhs=wv_t[:, kk, :],
                             start=(kk == 0), stop=(kk == CT - 1))
        nc.vector.tensor_copy(out=vaug_t[s][:, :, 0:CH],
                              in_=ps.rearrange("p (h e) -> p h e", e=CH))

    for oc in range(2):
        for t2 in range(TC):
            qk_unit(oc, t2)
    # (pair, ss) -> work units; vT for s must land by pair0 step s (wv needs
    # vaug[s] at step s+DELAY); qk for pair p anywhere before pair p
    sched = {}
    for s in range(ST):
        sched.setdefault((0, s), []).append(lambda s=s: vt_unit(s))
    for j, (oc, t2) in enumerate([(oc, t2) for oc in (2, 3) for t2 in range(TC)]):
        sched.setdefault((0, 4 + j), []).append(lambda oc=oc, t2=t2: qk_unit(oc, t2))
    for j, (oc, t2) in enumerate([(oc, t2) for oc in (4, 5) for t2 in range(TC)]):
        sched.setdefault((1, j), []).append(lambda oc=oc, t2=t2: qk_unit(oc, t2))
    for j, (oc, t2) in enumerate([(oc, t2) for oc in (6, 7) for t2 in range(TC)]):
        sched.setdefault((2, j), []).append(lambda oc=oc, t2=t2: qk_unit(oc, t2))

    # ---- attention: head pairs, wv DELAY steps behind scores/exp ----
    def division(p, hu_ps, htmp):
        """h = hu/D for pair p, into h_t[p].

        1/D on DVE costs 8 cycles/element of FREE dim, so the four D rows
        (4 x 512) are DMA-reshaped to (64,32) first: one 0.4us reciprocal.
        The reciprocal then takes a DRAM round trip so a partition-broadcast
        DMA (stride-0 partition reads are DRAM-only) can replicate each
        512-vector across 64 partitions for the final psum*sbuf multiply.
        Nothing lands on the scalar engine (pacing exp) or PE.
        """
        chains = [(half, t2) for half in range(2) for t2 in range(TC)]
        scr = DR.tile([4, 512], F32, name="scr", tag="scr", bufs=2)
        scr2 = DR.tile([4, 512], F32, name="scr2", tag="scr2", bufs=2)
        dall = W.tile([64, 32], F32, name="dall", tag="dall", bufs=2)
        ralt = W.tile([64, 32], F32, name="ralt", tag="ralt", bufs=2)
        for j, (half, t2) in enumerate(chains):
            dsb = W.tile([128, 512], F32, name="dsb", tag="dsb", bufs=4)
            nc.vector.tensor_copy(out=dsb[CH:CH + 1, :],
                                  in_=hu_ps[half][t2][CH:CH + 1, :])
            nc.sync.dma_start(out=scr[j:j + 1, :], in_=dsb[CH:CH + 1, :])
        nc.sync.dma_start(out=dall,
                          in_=scr.rearrange("a (b c) -> (a b) c", c=32))
        nc.vector.reciprocal(out=ralt, in_=dall)
        nc.sync.dma_start(out=scr2.rearrange("a (b c) -> (a b) c", c=32),
                          in_=ralt)
        for j, (half, t2) in enumerate(chains):
            fs = slice(t2 * 512, t2 * 512 + 512)
            rcb = W.tile([64, 512], F32, name="rcb", tag="rcb", bufs=4)
            row = scr2[j:j + 1, :]
            bcast = bass.AP(tensor=row.tensor, offset=row.offset,
                            ap=[[0, 64], list(row.ap[-1])])
            nc.sync.dma_start(out=rcb, in_=bcast)
            dst = h_t[p] if half == 0 else htmp
            with nc.allow_low_precision(reason="f32r tag on h; same payload"):
                nc.vector.tensor_mul(out=dst[0:CH, fs],
                                     in0=hu_ps[half][t2][0:CH, :],
                                     in1=rcb)
        # odd head's h goes to partitions 64..127 (cross-partition -> DMA)
        nc.sync.dma_start(out=h_t[p][64:128, :], in_=htmp[0:64, :])

    pend = None  # (p, hu_ps, htmp) awaiting division
    for p in range(4):
        hu_ps = [[None] * TC for _ in range(2)]
        htmp = W.tile([64, T], F32R, name="htmp", tag="htmp", bufs=2)
        expw = {}
        for ss in range(ST):
            scps = [None, None]
            for half in range(2):
                base = 64 * half
                sc = PS.tile([128, T], F32, name=f"scps{half}", tag="sc")
                scps[half] = sc
                for t2 in range(TC):
                    fs = slice(t2 * 512, t2 * 512 + 512)
                    nc.tensor.matmul(
                        out=sc[:, fs],
                        lhsT=k_t[p][base:base + 64, ss * 128:(ss + 1) * 128],
                        rhs=q_t[p][base:base + 64, fs],
                        start=True, stop=True)
                ew = W.tile([128, T], BF16, name="expw", tag="expw", bufs=10)
                nc.scalar.activation(out=ew, in_=scps[half],
                                     func=mybir.ActivationFunctionType.Exp,
                                     scale=0.125)
                expw[half, ss] = ew
            for unit in sched.get((p, ss), []):
                unit()
            if ss == 0 and pend is not None:
                division(*pend)
                pend = None
            if ss >= DELAY:
                _wv(nc, PS, vaug_t, expw, hu_ps, p, ss - DELAY)
        for ss in range(ST - DELAY, ST):
            _wv(nc, PS, vaug_t, expw, hu_ps, p, ss)
        pend = (p, hu_ps, htmp)
    division(*pend)

    # ---- proj + residual ----
    for oc in range(CT):
        ps = PS.tile([128, T], F32, name="prps", tag="sc")
        for t2 in range(TC):
            fs = slice(t2 * 512, t2 * 512 + 512)
            for kk in range(CT):
                nc.tensor.matmul(out=ps[:, fs],
                                 lhsT=wp_t[:, kk, oc * 128:(oc + 1) * 128],
                                 rhs=h_t[kk][:, fs],
                                 start=(kk == 0), stop=(kk == CT - 1))
        if use_bproj:
            prtmp = W.tile([128, T], F32, name="prtmp", tag="prtmp", bufs=2)
            nc.vector.tensor_scalar(out=prtmp, in0=ps, scalar1=bproj_sb[:, oc:oc + 1],
                                    scalar2=None, op0=mybir.AluOpType.add)
            with nc.allow_low_precision(reason="f32r tag on out; same payload"):
                nc.vector.tensor_add(out=x_t[oc], in0=x_t[oc], in1=prtmp)
        else:
            with nc.allow_low_precision(reason="f32r tag on out; same payload"):
                nc.vector.tensor_add(out=x_t[oc], in0=x_t[oc], in1=ps)
        nc.sync.dma_start(out=out_d[oc * 128:(oc + 1) * 128, :], in_=x_t[oc])


def _wv(nc, PS, vaug_t, expw, hu_ps, p, ss):
    for half in range(2):
        h = 2 * p + half
        for t2 in range(TC):
            if ss == 0:
                hu_ps[half][t2] = PS.tile([128, 512], F32,
                                          name="hups", tag="hu", bufs=4)
            fs = slice(t2 * 512, t2 * 512 + 512)
            nc.tensor.matmul(out=hu_ps[half][t2][0:CH + 1, :],
                             lhsT=vaug_t[ss][:, h, :],
                             rhs=expw[half, ss][:, fs],
                             start=(ss == 0), stop=(ss == ST - 1))


_PROGRAM_CACHE = {}


def _get_program(use_bqk, use_bproj):
    key = (use_bqk, use_bproj)
    if key not in _PROGRAM_CACHE:
        _PROGRAM_CACHE[key] = build_program(*key)
    return _PROGRAM_CACHE[key]


def make_host_inputs(x, gamma, beta, w_qkv, b_qkv, w_proj, b_proj):
    """Host-side preprocessing shared by all cores."""
    x = np.asarray(x, np.float32)
    w_qkv = np.asarray(w_qkv, np.float32)
    b_qkv = np.asarray(b_qkv, np.float32)
    w_proj = np.asarray(w_proj, np.float32)
    b_proj = np.asarray(b_proj, np.float32)
    gamma = np.asarray(gamma, np.float32)
    beta = np.asarray(beta, np.float32)

    # per-head slices of w_qkv rows (3c, c): head h -> q,k,v at 192h+{0,64,128}
    wq = np.stack([w_qkv[192 * h:192 * h + 64] for h in range(NH)])
    wk = np.stack([w_qkv[192 * h + 64:192 * h + 128] for h in range(NH)])
    wv = np.stack([w_qkv[192 * h + 128:192 * h + 192] for h in range(NH)])
    bq = np.stack([b_qkv[192 * h:192 * h + 64] for h in range(NH)])
    bk = np.stack([b_qkv[192 * h + 64:192 * h + 128] for h in range(NH)])
    bv = np.stack([b_qkv[192 * h + 128:192 * h + 192] for h in range(NH)])

    # wqk (512c, 1024): chunk 2p = q of heads (2p,2p+1), chunk 2p+1 = k of same
    chunks, bqk_chunks = [], []
    for p in range(4):
        chunks.append(np.concatenate([wq[2 * p], wq[2 * p + 1]], 0).T)
        chunks.append(np.concatenate([wk[2 * p], wk[2 * p + 1]], 0).T)
        bqk_chunks.append(np.concatenate([bq[2 * p], bq[2 * p + 1]], 0))
        bqk_chunks.append(np.concatenate([bk[2 * p], bk[2 * p + 1]], 0))
    wqk_host = np.concatenate(chunks, axis=1)                     # (512,1024)
    bqk_host = np.stack(bqk_chunks, axis=1)                       # (128,8)

    wvT_host = wv.reshape(C, C).T.copy()                          # (512c, 512vch)
    wprojT_host = w_proj.T.copy()                                 # (512c, 512o)
    # v-bias contributes exactly b_v through the softmax (rows sum to 1);
    # fold it into the proj bias
    bproj_eff = b_proj + w_proj @ bv.reshape(C)
    bproj_host = bproj_eff.reshape(CT, 128).T.copy()
    gamma_host = gamma.reshape(CT, 128).T.copy()
    beta_host = beta.reshape(CT, 128).T.copy()

    cidx = np.arange(C)
    # gsel aggregates raw [sum, sum(x^2)] rows -> per-group means
    gsel_host = (cidx[:, None] // GS == np.arange(NG)[None, :]).astype(np.float32) / (GS * T)
    gselt_host = (cidx[None, :] // GS == np.arange(NG)[:, None]).astype(np.float32)

    def ktile(a):
        # (512, N) -> (128, 4, N): partition-major layout for one big tile
        return np.ascontiguousarray(a.reshape(CT, 128, -1).transpose(1, 0, 2))

    use_bqk = bool(np.any(bqk_host))
    use_bproj = bool(np.any(bproj_host))
    biases = np.concatenate([bqk_host, bproj_host, gamma_host, beta_host], axis=1)

    common = {
        "wqk": ktile(wqk_host),
        "wv": ktile(wvT_host),
        "wproj": ktile(wprojT_host),
        "gsel": ktile(gsel_host),
        "gselt": np.ascontiguousarray(gselt_host),
        "biases": np.ascontiguousarray(biases.astype(np.float32)),
    }
    return x, common, use_bqk, use_bproj


def kernel(x, gamma, beta, w_qkv, b_qkv, w_proj, b_proj):
    b, c, H, Wd = x.shape
    assert (b, c, H, Wd) == (8, C, 32, 32)
    xf, common, use_bqk, use_bproj = make_host_inputs(
        x, gamma, beta, w_qkv, b_qkv, w_proj, b_proj)
    xf = xf.reshape(b, C, T)

    nc = _get_program(use_bqk, use_bproj)
    if not getattr(nc, "_waits_split", False):
        _split_multi_waits(nc)
        nc._waits_split = True
    in_maps = [dict(common, x=np.ascontiguousarray(xf[i])) for i in range(NCORES)]
    res = run_bass_kernel_spmd(nc, in_maps, list(range(NCORES)))
    out = np.stack([res.results[i]["out"] for i in range(NCORES)])
    return out.reshape(b, C, H, Wd).astype(np.float32)


if __name__ == "__main__":
    rng = np.random.default_rng(0)
    args = {
        "x": rng.standard_normal((8, C, 32, 32), dtype=np.float32),
        "gamma": np.ones(C, np.float32),
        "beta": np.zeros(C, np.float32),
        "w_qkv": (rng.standard_normal((3 * C, C)) * 0.02).astype(np.float32),
        "b_qkv": np.zeros(3 * C, np.float32),
        "w_proj": (rng.standard_normal((C, C)) * 0.02).astype(np.float32),
        "b_proj": np.zeros(C, np.float32),
    }
    out = kernel(**args)
    print(out.shape, out.dtype)


# revision 26
# speedup vs baseline: 1.0810x; 1.0810x over previous
"""AttentionBlock kernel for 8 Trainium2 NeuronCores.

Problem: x(8,512,32,32) -> GroupNorm(32) -> qkv 1x1 conv -> 8-head attention
         over T=1024 tokens -> proj 1x1 conv -> residual.

Sharding: pure data parallel - one batch element per core, no collectives.

Per-core dataflow (c=512 channels on partitions, T=1024 tokens on free dim):
  1. GroupNorm stats: bn_stats per channel; tiny PE matmuls (group-selector
     matrices) aggregate across the 16 channels of each group and broadcast
     group stats back to channels; xn = x*A + B via one tensor_scalar per tile.
  2. q,k projection (fp32r) with head-pair-permuted weights; outputs cast to
     bf16: q/k of heads (2p,2p+1) stacked on partitions 0-63/64-127 of one
     (128,1024) tile each, so the K=64 score matmuls of a pair land in
     disjoint PE row groups.
  3. v is produced TRANSPOSED directly by the matmul vT = xn^T @ wvT
     (lhsT = xn), avoiding any explicit transpose for the second attention
     matmul; cast to bf16 with an all-ones column appended (M=65) so the WV
     matmul also emits the softmax denominator D[t] as output row 64.
  4. scores^T(s,t) = k^T q per head in bf16 (hw runs K<128 fp32r matmuls at
     half rate; bf16 runs at full rate and the ~4e-3 rounding is well within
     tolerance).
  5. exp on the scalar engine reading PSUM, writing bf16 (scale=0.125 folds
     the attention scaling exactly); WV accumulates hu (and D at row 64) in
     fp32 PSUM. 1/D via reciprocal_approx_fast after a K=1 ones-matmul
     broadcasts D to all partitions.
  6. proj (fp32r) + residual into the x tiles, DMA out.

Schedule: WV runs 3 pipeline steps behind scores/exp so the scalar engine
never starves; the previous pair's division chain is emitted inside the next
pair (keeps the in-order PE queue from stalling on it); qk chunks 2..7 and
all vT matmuls are spread across pair 0's pipeline steps.
"""

import sys

for _p in ("/opt/trn_rl_repo",):
    if _p not in sys.path:
        sys.path.insert(0, _p)

import numpy as np

import concourse.bass as bass
import concourse.tile as tile
from concourse import mybir
from concourse.bass_utils import run_bass_kernel_spmd
from concourse.vector_clock import ScopedClock, VectorClock


def _patched_drain_and_barrier(self, tick_clock, wait_clock):
    # This container's walrus rejects instructions carrying more than one
    # sync wait. Split the final drain's global-clock waits across NOPs.
    g = tick_clock.global_clock
    n = len(g)
    for lo in range(0, n, 4):
        vec = [g[p] if lo <= p < lo + 4 else 0 for p in range(n)]
        if not any(vec):
            continue
        nop_inst = self.nc.sync.nop()
        wait_clock.add_sem_waits(nop_inst.ins, ScopedClock({None: VectorClock(vec)}))
    self.nc.sync.drain()
    self.nc.all_engine_barrier()
    assert self.sems is not None
    popped = self.nc._tile_sem_poison_stack.pop()
    assert popped is self._sem_poison
    self.nc.clear_and_free_semaphores(list(self.sems.allocated().values()))
    self.nc.all_engine_barrier()


tile.TileContext._drain_and_barrier = _patched_drain_and_barrier


def _split_multi_waits(nc):
    # This walrus build accepts at most one sync wait per instruction. Hoist
    # surplus waits onto same-engine NOPs placed immediately before.
    n = 0
    for fn in nc.m.functions:
        for blk in fn.blocks:
            out = []
            for inst in blk.instructions:
                si = inst.sync_info
                if si is not None and si.on_wait and len(si.on_wait) > 1:
                    waits = list(si.on_wait)
                    for w in waits[:-1]:
                        nop = mybir.InstNoOp(name=f"{inst.name}_w{n}", ins=[], outs=[])
                        n += 1
                        nop.engine = inst.engine
                        nop.sync_info = mybir.SyncInfo(on_wait=[w], on_update=[])
                        out.append(nop)
                    si.on_wait = [waits[-1]]
                out.append(inst)
            blk.instructions = out


F32 = mybir.dt.float32
F32R = mybir.dt.float32r
BF16 = mybir.dt.bfloat16

C = 512
T = 1024
NH = 8          # heads
CH = C // NH    # 64 channels per head
NG = 32         # groups
GS = C // NG    # 16 channels per group
EPS = 1e-5
NCORES = 8
CT = C // 128   # 4 channel tiles
ST = T // 128   # 8 s tiles
TC = T // 512   # 2 t chunks
DELAY = 3       # wv runs this many pipeline steps behind scores/exp


def build_program(use_bqk: bool, use_bproj: bool) -> bass.Bass:
    nc = bass.Bass()

    x_d = nc.dram_tensor("x", [C, T], F32R, kind="ExternalInput")
    wqk_d = nc.dram_tensor("wqk", [128, CT, 2 * C], F32R, kind="ExternalInput")
    wv_d = nc.dram_tensor("wv", [128, CT, C], F32R, kind="ExternalInput")
    wproj_d = nc.dram_tensor("wproj", [128, CT, C], F32R, kind="ExternalInput")
    gsel_d = nc.dram_tensor("gsel", [128, CT, NG], F32R, kind="ExternalInput")
    gselt_d = nc.dram_tensor("gselt", [NG, C], F32R, kind="ExternalInput")
    # combined small params: [bqk(8) | bproj(4) | gamma(4) | beta(4)]
    bias_d = nc.dram_tensor("biases", [128, 20], F32, kind="ExternalInput")
    out_d = nc.dram_tensor("out", [C, T], F32R, kind="ExternalOutput")

    with tile.TileContext(nc) as tc:
        with (
            tc.tile_pool(name="persist", bufs=1) as P,
            tc.tile_pool(name="work", bufs=2) as W,
            tc.tile_pool(name="ps", bufs=2, space="PSUM") as PS,
            tc.tile_pool(name="dr", bufs=2, space="DRAM") as DR,
        ):
            _body(nc, tc, P, W, PS, DR, locals(), use_bqk, use_bproj)
    return nc


def _body(nc, tc, P, W, PS, DR, d, use_bqk, use_bproj):
    x_d, wqk_d, wv_d, wproj_d = d["x_d"], d["wqk_d"], d["wv_d"], d["wproj_d"]
    gsel_d, gselt_d, bias_d, out_d = d["gsel_d"], d["gselt_d"], d["bias_d"], d["out_d"]

    # ---- persistent SBUF tiles ----
    x_t = [P.tile([128, T], F32R, name=f"x{i}", tag=f"x{i}") for i in range(CT)]
    xn_t = [P.tile([128, T], F32R, name=f"xn{i}", tag=f"xn{i}") for i in range(CT)]
    h_t = [P.tile([128, T], F32R, name=f"h{i}", tag=f"h{i}") for i in range(CT)]
    q_t = [P.tile([128, T], BF16, name=f"q{p}", tag=f"q{p}") for p in range(4)]
    k_t = [P.tile([128, T], BF16, name=f"k{p}", tag=f"k{p}") for p in range(4)]
    vaug_t = [P.tile([128, NH, CH + 1], BF16, name=f"va{s}", tag=f"va{s}") for s in range(ST)]
    wqk_t = P.tile([128, CT, 2 * C], F32R, name="wqk_t")
    wv_t = P.tile([128, CT, C], F32R, name="wv_t")
    wp_t = P.tile([128, CT, C], F32R, name="wp_t")
    gsel_t = P.tile([128, CT, NG], F32R, name="gsel_t")
    gselt_t = P.tile([NG, C], F32R, name="gselt_t")
    ab_t = [P.tile([128, 2], F32, name=f"ab{i}", tag=f"ab{i}") for i in range(CT)]
    bias_sb = P.tile([128, 20], F32, name="bias_sb")
    bqk_sb = bias_sb[:, 0:8]
    bproj_sb = bias_sb[:, 8:12]
    gamma_sb = bias_sb[:, 12:16]
    beta_sb = bias_sb[:, 16:20]
    eps_sb = P.tile([NG, 1], F32, name="eps_sb")
    mvr_sb = P.tile([NG, 2], F32R, name="mvr_sb")

    # ---- input DMAs: x on sync, weights on the gpsimd SWDGE queues ----
    # (separate semaphore pool, so a big weight transfer never blocks an
    # x tile's HWDGE slot)
    x_dmas = []
    for i in range(CT):
        for half in range(2):
            fs = slice(half * 512, half * 512 + 512)
            x_dmas.append(nc.sync.dma_start(
                out=x_t[i][:, fs], in_=x_d[i * 128:(i + 1) * 128, fs]))
    nc.gpsimd.dma_start(out=gsel_t, in_=gsel_d[:, :, :])
    nc.gpsimd.dma_start(out=gselt_t, in_=gselt_d[:, :])
    nc.gpsimd.dma_start(out=bias_sb, in_=bias_d[:, :])
    # weight transfers start only after x has landed: stats (and the whole
    # pipeline behind them) need x first, and HBM bandwidth is the startup
    # bottleneck. wqk is split so pair 0's q/k columns land first.
    w_dmas = [
        nc.gpsimd.dma_start(out=wqk_t[:, :, 0:256], in_=wqk_d[:, :, 0:256]),
        nc.gpsimd.dma_start(out=wqk_t[:, :, 256:2 * C], in_=wqk_d[:, :, 256:2 * C]),
        nc.gpsimd.dma_start(out=wv_t, in_=wv_d[:, :, :]),
        nc.gpsimd.dma_start(out=wp_t, in_=wproj_d[:, :, :]),
    ]
    for w in w_dmas:
        bass._add_dep_helper(w.ins, x_dmas[-1].ins, sync=True,
                             reason="weights yield HBM bandwidth to x")
    nc.vector.memset(eps_sb, EPS)

    # ---- phase 1: group norm statistics ----
    # per-channel sum via DVE tensor_scalar accumulate (dumps into xn, which
    # is fully overwritten later) and sum(x^2) via ACT Square accumulate
    # (dumps into h_t, dead until attention); both run while x halves land
    mv32_ps = PS.tile([NG, 2], F32, name="mv32_ps", tag="sc")
    for i in range(CT):
        st2 = W.tile([128, 2], F32R, name="st2", tag="st2", bufs=4)
        with nc.allow_low_precision(reason="f32r accum tags; same 4B payload"):
            nc.vector.tensor_scalar(out=xn_t[i], in0=x_t[i],
                                    scalar1=1.0, scalar2=0.0,
                                    op0=mybir.AluOpType.mult,
                                    op1=mybir.AluOpType.add,
                                    accum_out=st2[:, 0:1])
            nc.scalar.activation(out=h_t[i], in_=x_t[i],
                                 func=mybir.ActivationFunctionType.Square,
                                 accum_out=st2[:, 1:2])
        # group aggregation: (1/(16*1024)) * indicator^T @ [sum, sumsq]
        nc.tensor.matmul(out=mv32_ps, lhsT=gsel_t[:, i, :], rhs=st2,
                         start=(i == 0), stop=(i == CT - 1))
    mv32_sb = W.tile([NG, 2], F32, name="mv32_sb", tag="mv32", bufs=1)
    sdtmp = W.tile([NG, 2], F32, name="sdtmp", tag="sdtmp", bufs=1)
    nc.vector.tensor_copy(out=mv32_sb, in_=mv32_ps)
    # var_g = E[x^2]_g - mean_g^2 ; rstd = 1/sqrt(var+eps)
    nc.vector.tensor_mul(out=sdtmp[:, 0:1], in0=mv32_sb[:, 0:1], in1=mv32_sb[:, 0:1])
    nc.vector.tensor_sub(out=sdtmp[:, 1:2], in0=mv32_sb[:, 1:2], in1=sdtmp[:, 0:1])
    nc.scalar.activation(out=sdtmp[:, 1:2], in_=sdtmp[:, 1:2],
                         func=mybir.ActivationFunctionType.Sqrt,
                         bias=eps_sb, scale=1.0)
    with nc.allow_low_precision(reason="f32r tag on rstd; same 4-byte payload"):
        nc.vector.reciprocal(out=mvr_sb[:, 1:2], in_=sdtmp[:, 1:2])
    nc.vector.tensor_copy(out=mvr_sb[:, 0:1], in_=mv32_sb[:, 0:1])
    # broadcast group stats back to channels, per-channel A/B, xn = x*A + B
    for i in range(CT):
        mr_ps = PS.tile([128, 2], F32, name="mr_ps", tag="sc")
        nc.tensor.matmul(out=mr_ps, lhsT=gselt_t[:, i * 128:(i + 1) * 128],
                         rhs=mvr_sb, start=True, stop=True)
        abm = W.tile([128, 1], F32, name="abm", tag="abm", bufs=4)
        nc.vector.tensor_mul(out=ab_t[i][:, 0:1], in0=mr_ps[:, 1:2], in1=gamma_sb[:, i:i + 1])
        nc.vector.tensor_mul(out=abm, in0=mr_ps[:, 0:1], in1=ab_t[i][:, 0:1])
        nc.vector.tensor_sub(out=ab_t[i][:, 1:2], in0=beta_sb[:, i:i + 1], in1=abm)
        nc.vector.tensor_scalar(out=xn_t[i], in0=x_t[i],
                                scalar1=ab_t[i][:, 0:1], scalar2=ab_t[i][:, 1:2],
                                op0=mybir.AluOpType.mult, op1=mybir.AluOpType.add)

    # ---- deferred PE work: 4-matmul units spread across pairs 0..2 ----
    def qk_unit(oc, t2):
        p, is_k = oc // 2, oc % 2
        fs = slice(t2 * 512, t2 * 512 + 512)
        ps = PS.tile([128, 512], F32, name="qkps", tag="sc")
        for kk in range(CT):
            nc.tensor.matmul(out=ps,
                             lhsT=wqk_t[:, kk, oc * 128:(oc + 1) * 128],
                             rhs=xn_t[kk][:, fs],
                             start=(kk == 0), stop=(kk == CT - 1))
        dst = (k_t[p] if is_k else q_t[p])[:, fs]
        if use_bqk:
            nc.vector.tensor_scalar(out=dst, in0=ps, scalar1=bqk_sb[:, oc:oc + 1],
                                    scalar2=None, op0=mybir.AluOpType.add)
        else:
            nc.vector.tensor_copy(out=dst, in_=ps)

    def vt_unit(s):
        # whole-tile memset (strided ones-column memset fails this walrus);
        # the copy below overwrites cols 0..63 per head, col 64 stays 1.0
        nc.vector.memset(vaug_t[s], 1.0)
        ps = PS.tile([128, C], F32, name="vtps", tag="sc")
        for kk in range(CT):
            nc.tensor.matmul(out=ps,
                             lhsT=xn_t[kk][:, s * 128:(s + 1) * 128],
                             rhs=wv_t[:, kk, :],
                             start=(kk == 0), stop=(kk == CT - 1))
        nc.vector.tensor_copy(out=vaug_t[s][:, :, 0:CH],
                              in_=ps.rearrange("p (h e) -> p h e", e=CH))

    for oc in range(2):
        for t2 in range(TC):
            qk_unit(oc, t2)
    # (pair, ss) -> work units; vT for s must land by pair0 step s (wv needs
    # vaug[s] at step s+DELAY); qk for pair p anywhere before pair p
    sched = {}
    for s in range(ST):
        sched.setdefault((0, s), []).append(lambda s=s: vt_unit(s))
    for j, (oc, t2) in enumerate([(oc, t2) for oc in (2, 3) for t2 in range(TC)]):
        sched.setdefault((0, 4 + j), []).append(lambda oc=oc, t2=t2: qk_unit(oc, t2))
    for j, (oc, t2) in enumerate([(oc, t2) for oc in (4, 5) for t2 in range(TC)]):
        sched.setdefault((1, j), []).append(lambda oc=oc, t2=t2: qk_unit(oc, t2))
    for j, (oc, t2) in enumerate([(oc, t2) for oc in (6, 7) for t2 in range(TC)]):
        sched.setdefault((2, j), []).append(lambda oc=oc, t2=t2: qk_unit(oc, t2))

    # ---- attention: head pairs, wv DELAY steps behind scores/exp ----
    def division(p, hu_ps, htmp):
        """h = hu/D for pair p, into h_t[p].

        1/D on DVE costs 8 cycles/element of FREE dim, so the four D rows
        (4 x 512) are DMA-reshaped to (64,32) first: one 0.4us reciprocal.
        The reciprocal then takes a DRAM round trip so a partition-broadcast
        DMA (stride-0 partition reads are DRAM-only) can replicate each
        512-vector across 64 partitions for the final psum*sbuf multiply.
        Nothing lands on the scalar engine (pacing exp) or PE.
        """
        chains = [(half, t2) for half in range(2) for t2 in range(TC)]
        scr = DR.tile([4, 512], F32, name="scr", tag="scr", bufs=2)
        scr2 = DR.tile([4, 512], F32, name="scr2", tag="scr2", bufs=2)
        dall = W.tile([64, 32], F32, name="dall", tag="dall", bufs=2)
        ralt = W.tile([64, 32], F32, name="ralt", tag="ralt", bufs=2)
        for j, (half, t2) in enumerate(chains):
            dsb = W.tile([128, 512], F32, name="dsb", tag="dsb", bufs=4)
            nc.vector.tensor_copy(out=dsb[CH:CH + 1, :],
                                  in_=hu_ps[half][t2][CH:CH + 1, :])
            nc.sync.dma_start(out=scr[j:j + 1, :], in_=dsb[CH:CH + 1, :])
        nc.sync.dma_start(out=dall,
                          in_=scr.rearrange("a (b c) -> (a b) c", c=32))
        nc.vector.reciprocal(out=ralt, in_=dall)
        nc.sync.dma_start(out=scr2.rearrange("a (b c) -> (a b) c", c=32),
                          in_=ralt)
        for j, (half, t2) in enumerate(chains):
            fs = slice(t2 * 512, t2 * 512 + 512)
            rcb = W.tile([64, 512], F32, name="rcb", tag="rcb", bufs=4)
            row = scr2[j:j + 1, :]
            bcast = bass.AP(tensor=row.tensor, offset=row.offset,
                            ap=[[0, 64], list(row.ap[-1])])
            nc.sync.dma_start(out=rcb, in_=bcast)
            dst = h_t[p] if half == 0 else htmp
            with nc.allow_low_precision(reason="f32r tag on h; same payload"):
                nc.vector.tensor_mul(out=dst[0:CH, fs],
                                     in0=hu_ps[half][t2][0:CH, :],
                                     in1=rcb)
        # odd head's h goes to partitions 64..127 (cross-partition -> DMA)
        nc.sync.dma_start(out=h_t[p][64:128, :], in_=htmp[0:64, :])

    pend = None  # (p, hu_ps, htmp) awaiting division
    for p in range(4):
        hu_ps = [[None] * TC for _ in range(2)]
        htmp = W.tile([64, T], F32R, name="htmp", tag="htmp", bufs=2)
        expw = {}
        for ss in range(ST):
            scps = [None, None]
            for half in range(2):
                base = 64 * half
                sc = PS.tile([128, T], F32, name=f"scps{half}", tag="sc")
                scps[half] = sc
                for t2 in range(TC):
                    fs = slice(t2 * 512, t2 * 512 + 512)
                    nc.tensor.matmul(
                        out=sc[:, fs],
                        lhsT=k_t[p][base:base + 64, ss * 128:(ss + 1) * 128],
                        rhs=q_t[p][base:base + 64, fs],
                        start=True, stop=True)
                ew = W.tile([128, T], BF16, name="expw", tag="expw", bufs=10)
                nc.scalar.activation(out=ew, in_=scps[half],
                                     func=mybir.ActivationFunctionType.Exp,
                                     scale=0.125)
                expw[half, ss] = ew
            for unit in sched.get((p, ss), []):
                unit()
            if ss == 0 and pend is not None:
                division(*pend)
                pend = None
            if ss >= DELAY:
                _wv(nc, PS, vaug_t, expw, hu_ps, p, ss - DELAY)
        for ss in range(ST - DELAY, ST):
            _wv(nc, PS, vaug_t, expw, hu_ps, p, ss)
        pend = (p, hu_ps, htmp)
    division(*pend)

    # ---- proj + residual ----
    for oc in range(CT):
        ps = PS.tile([128, T], F32, name="prps", tag="sc")
        for t2 in range(TC):
            fs = slice(t2 * 512, t2 * 512 + 512)
            for kk in range(CT):
                nc.tensor.matmul(out=ps[:, fs],
                                 lhsT=wp_t[:, kk, oc * 128:(oc + 1) * 128],
                                 rhs=h_t[kk][:, fs],
                                 start=(kk == 0), stop=(kk == CT - 1))
        if use_bproj:
            prtmp = W.tile([128, T], F32, name="prtmp", tag="prtmp", bufs=2)
            nc.vector.tensor_scalar(out=prtmp, in0=ps, scalar1=bproj_sb[:, oc:oc + 1],
                                    scalar2=None, op0=mybir.AluOpType.add)
            with nc.allow_low_precision(reason="f32r tag on out; same payload"):
                nc.vector.tensor_add(out=x_t[oc], in0=x_t[oc], in1=prtmp)
        else:
            with nc.allow_low_precision(reason="f32r tag on out; same payload"):
                nc.vector.tensor_add(out=x_t[oc], in0=x_t[oc], in1=ps)
        nc.sync.dma_start(out=out_d[oc * 128:(oc + 1) * 128, :], in_=x_t[oc])


def _wv(nc, PS, vaug_t, expw, hu_ps, p, ss):
    for half in range(2):
        h = 2 * p + half
        for t2 in range(TC):
            if ss == 0:
                hu_ps[half][t2] = PS.tile([128, 512], F32,
                                          name="hups", tag="hu", bufs=4)
            fs = slice(t2 * 512, t2 * 512 + 512)
            nc.tensor.matmul(out=hu_ps[half][t2][0:CH + 1, :],
                             lhsT=vaug_t[ss][:, h, :],
                             rhs=expw[half, ss][:, fs],
                             start=(ss == 0), stop=(ss == ST - 1))


_PROGRAM_CACHE = {}


def _get_program(use_bqk, use_bproj):
    key = (use_bqk, use_bproj)
    if key not in _PROGRAM_CACHE:
        _PROGRAM_CACHE[key] = build_program(*key)
    return _PROGRAM_CACHE[key]


def make_host_inputs(x, gamma, beta, w_qkv, b_qkv, w_proj, b_proj):
    """Host-side preprocessing shared by all cores."""
    x = np.asarray(x, np.float32)
    w_qkv = np.asarray(w_qkv, np.float32)
    b_qkv = np.asarray(b_qkv, np.float32)
    w_proj = np.asarray(w_proj, np.float32)
    b_proj = np.asarray(b_proj, np.float32)
    gamma = np.asarray(gamma, np.float32)
    beta = np.asarray(beta, np.float32)

    # per-head slices of w_qkv rows (3c, c): head h -> q,k,v at 192h+{0,64,128}
    wq = np.stack([w_qkv[192 * h:192 * h + 64] for h in range(NH)])
    wk = np.stack([w_qkv[192 * h + 64:192 * h + 128] for h in range(NH)])
    wv = np.stack([w_qkv[192 * h + 128:192 * h + 192] for h in range(NH)])
    bq = np.stack([b_qkv[192 * h:192 * h + 64] for h in range(NH)])
    bk = np.stack([b_qkv[192 * h + 64:192 * h + 128] for h in range(NH)])
    bv = np.stack([b_qkv[192 * h + 128:192 * h + 192] for h in range(NH)])

    # wqk (512c, 1024): chunk 2p = q of heads (2p,2p+1), chunk 2p+1 = k of same
    chunks, bqk_chunks = [], []
    for p in range(4):
        chunks.append(np.concatenate([wq[2 * p], wq[2 * p + 1]], 0).T)
        chunks.append(np.concatenate([wk[2 * p], wk[2 * p + 1]], 0).T)
        bqk_chunks.append(np.concatenate([bq[2 * p], bq[2 * p + 1]], 0))
        bqk_chunks.append(np.concatenate([bk[2 * p], bk[2 * p + 1]], 0))
    wqk_host = np.concatenate(chunks, axis=1)                     # (512,1024)
    bqk_host = np.stack(bqk_chunks, axis=1)                       # (128,8)

    wvT_host = wv.reshape(C, C).T.copy()                          # (512c, 512vch)
    wprojT_host = w_proj.T.copy()                                 # (512c, 512o)
    # v-bias contributes exactly b_v through the softmax (rows sum to 1);
    # fold it into the proj bias
    bproj_eff = b_proj + w_proj @ bv.reshape(C)
    bproj_host = bproj_eff.reshape(CT, 128).T.copy()
    gamma_host = gamma.reshape(CT, 128).T.copy()
    beta_host = beta.reshape(CT, 128).T.copy()

    cidx = np.arange(C)
    # gsel aggregates raw [sum, sum(x^2)] rows -> per-group means
    gsel_host = (cidx[:, None] // GS == np.arange(NG)[None, :]).astype(np.float32) / (GS * T)
    gselt_host = (cidx[None, :] // GS == np.arange(NG)[:, None]).astype(np.float32)

    def ktile(a):
        # (512, N) -> (128, 4, N): partition-major layout for one big tile
        return np.ascontiguousarray(a.reshape(CT, 128, -1).transpose(1, 0, 2))

    use_bqk = bool(np.any(bqk_host))
    use_bproj = bool(np.any(bproj_host))
    biases = np.concatenate([bqk_host, bproj_host, gamma_host, beta_host], axis=1)

    common = {
        "wqk": ktile(wqk_host),
        "wv": ktile(wvT_host),
        "wproj": ktile(wprojT_host),
        "gsel": ktile(gsel_host),
        "gselt": np.ascontiguousarray(gselt_host),
        "biases": np.ascontiguousarray(biases.astype(np.float32)),
    }
    return x, common, use_bqk, use_bproj


def kernel(x, gamma, beta, w_qkv, b_qkv, w_proj, b_proj):
    b, c, H, Wd = x.shape
    assert (b, c, H, Wd) == (8, C, 32, 32)
    xf, common, use_bqk, use_bproj = make_host_inputs(
        x, gamma, beta, w_qkv, b_qkv, w_proj, b_proj)
    xf = xf.reshape(b, C, T)

    nc = _get_program(use_bqk, use_bproj)
    if not getattr(nc, "_waits_split", False):
        _split_multi_waits(nc)
        nc._waits_split = True
    in_maps = [dict(common, x=np.ascontiguousarray(xf[i])) for i in range(NCORES)]
    res = run_bass_kernel_spmd(nc, in_maps, list(range(NCORES)))
    out = np.stack([res.results[i]["out"] for i in range(NCORES)])
    return out.reshape(b, C, H, Wd).astype(np.float32)


if __name__ == "__main__":
    rng = np.random.default_rng(0)
    args = {
        "x": rng.standard_normal((8, C, 32, 32), dtype=np.float32),
        "gamma": np.ones(C, np.float32),
        "beta": np.zeros(C, np.float32),
        "w_qkv": (rng.standard_normal((3 * C, C)) * 0.02).astype(np.float32),
        "b_qkv": np.zeros(3 * C, np.float32),
        "w_proj": (rng.standard_normal((C, C)) * 0.02).astype(np.float32),
        "b_proj": np.zeros(C, np.float32),
    }
    out = kernel(**args)
    print(out.shape, out.dtype)


# revision 27
# speedup vs baseline: 1.0874x; 1.0060x over previous
"""AttentionBlock kernel for 8 Trainium2 NeuronCores.

Problem: x(8,512,32,32) -> GroupNorm(32) -> qkv 1x1 conv -> 8-head attention
         over T=1024 tokens -> proj 1x1 conv -> residual.

Sharding: pure data parallel - one batch element per core, no collectives.

Per-core dataflow (c=512 channels on partitions, T=1024 tokens on free dim):
  1. GroupNorm stats: bn_stats per channel; tiny PE matmuls (group-selector
     matrices) aggregate across the 16 channels of each group and broadcast
     group stats back to channels; xn = x*A + B via one tensor_scalar per tile.
  2. q,k projection (fp32r) with head-pair-permuted weights; outputs cast to
     bf16: q/k of heads (2p,2p+1) stacked on partitions 0-63/64-127 of one
     (128,1024) tile each, so the K=64 score matmuls of a pair land in
     disjoint PE row groups.
  3. v is produced TRANSPOSED directly by the matmul vT = xn^T @ wvT
     (lhsT = xn), avoiding any explicit transpose for the second attention
     matmul; cast to bf16 with an all-ones column appended (M=65) so the WV
     matmul also emits the softmax denominator D[t] as output row 64.
  4. scores^T(s,t) = k^T q per head in bf16 (hw runs K<128 fp32r matmuls at
     half rate; bf16 runs at full rate and the ~4e-3 rounding is well within
     tolerance).
  5. exp on the scalar engine reading PSUM, writing bf16 (scale=0.125 folds
     the attention scaling exactly); WV accumulates hu (and D at row 64) in
     fp32 PSUM. 1/D via reciprocal_approx_fast after a K=1 ones-matmul
     broadcasts D to all partitions.
  6. proj (fp32r) + residual into the x tiles, DMA out.

Schedule: WV runs 3 pipeline steps behind scores/exp so the scalar engine
never starves; the previous pair's division chain is emitted inside the next
pair (keeps the in-order PE queue from stalling on it); qk chunks 2..7 and
all vT matmuls are spread across pair 0's pipeline steps.
"""

import sys

for _p in ("/opt/trn_rl_repo",):
    if _p not in sys.path:
        sys.path.insert(0, _p)

import numpy as np

import concourse.bass as bass
import concourse.tile as tile
from concourse import mybir
from concourse.bass_utils import run_bass_kernel_spmd
from concourse.vector_clock import ScopedClock, VectorClock


def _patched_drain_and_barrier(self, tick_clock, wait_clock):
    # This container's walrus rejects instructions carrying more than one
    # sync wait. Split the final drain's global-clock waits across NOPs.
    g = tick_clock.global_clock
    n = len(g)
    for lo in range(0, n, 4):
        vec = [g[p] if lo <= p < lo + 4 else 0 for p in range(n)]
        if not any(vec):
            continue
        nop_inst = self.nc.sync.nop()
        wait_clock.add_sem_waits(nop_inst.ins, ScopedClock({None: VectorClock(vec)}))
    self.nc.sync.drain()
    self.nc.all_engine_barrier()
    assert self.sems is not None
    popped = self.nc._tile_sem_poison_stack.pop()
    assert popped is self._sem_poison
    self.nc.clear_and_free_semaphores(list(self.sems.allocated().values()))
    self.nc.all_engine_barrier()


tile.TileContext._drain_and_barrier = _patched_drain_and_barrier


def _split_multi_waits(nc):
    # This walrus build accepts at most one sync wait per instruction. Hoist
    # surplus waits onto same-engine NOPs placed immediately before.
    n = 0
    for fn in nc.m.functions:
        for blk in fn.blocks:
            out = []
            for inst in blk.instructions:
                si = inst.sync_info
                if si is not None and si.on_wait and len(si.on_wait) > 1:
                    waits = list(si.on_wait)
                    for w in waits[:-1]:
                        nop = mybir.InstNoOp(name=f"{inst.name}_w{n}", ins=[], outs=[])
                        n += 1
                        nop.engine = inst.engine
                        nop.sync_info = mybir.SyncInfo(on_wait=[w], on_update=[])
                        out.append(nop)
                    si.on_wait = [waits[-1]]
                out.append(inst)
            blk.instructions = out


F32 = mybir.dt.float32
F32R = mybir.dt.float32r
BF16 = mybir.dt.bfloat16

C = 512
T = 1024
NH = 8          # heads
CH = C // NH    # 64 channels per head
NG = 32         # groups
GS = C // NG    # 16 channels per group
EPS = 1e-5
NCORES = 8
CT = C // 128   # 4 channel tiles
ST = T // 128   # 8 s tiles
TC = T // 512   # 2 t chunks
DELAY = 3       # wv runs this many pipeline steps behind scores/exp


def build_program(use_bqk: bool, use_bproj: bool) -> bass.Bass:
    nc = bass.Bass()

    x_d = nc.dram_tensor("x", [C, T], F32R, kind="ExternalInput")
    wqk_d = nc.dram_tensor("wqk", [128, CT, 2 * C], F32R, kind="ExternalInput")
    wv_d = nc.dram_tensor("wv", [128, CT, C], F32R, kind="ExternalInput")
    wproj_d = nc.dram_tensor("wproj", [128, CT, C], F32R, kind="ExternalInput")
    gsel_d = nc.dram_tensor("gsel", [128, CT, NG], F32R, kind="ExternalInput")
    gselt_d = nc.dram_tensor("gselt", [NG, C], F32R, kind="ExternalInput")
    # combined small params: [bqk(8) | bproj(4) | gamma(4) | beta(4)]
    bias_d = nc.dram_tensor("biases", [128, 20], F32, kind="ExternalInput")
    out_d = nc.dram_tensor("out", [C, T], F32R, kind="ExternalOutput")

    with tile.TileContext(nc) as tc:
        with (
            tc.tile_pool(name="persist", bufs=1) as P,
            tc.tile_pool(name="work", bufs=2) as W,
            tc.tile_pool(name="ps", bufs=2, space="PSUM") as PS,
            tc.tile_pool(name="dr", bufs=2, space="DRAM") as DR,
        ):
            _body(nc, tc, P, W, PS, DR, locals(), use_bqk, use_bproj)
    return nc


def _body(nc, tc, P, W, PS, DR, d, use_bqk, use_bproj):
    x_d, wqk_d, wv_d, wproj_d = d["x_d"], d["wqk_d"], d["wv_d"], d["wproj_d"]
    gsel_d, gselt_d, bias_d, out_d = d["gsel_d"], d["gselt_d"], d["bias_d"], d["out_d"]

    # ---- persistent SBUF tiles ----
    x_t = [P.tile([128, T], F32R, name=f"x{i}", tag=f"x{i}") for i in range(CT)]
    xn_t = [P.tile([128, T], F32R, name=f"xn{i}", tag=f"xn{i}") for i in range(CT)]
    h_t = [P.tile([128, T], F32R, name=f"h{i}", tag=f"h{i}") for i in range(CT)]
    q_t = [P.tile([128, T], BF16, name=f"q{p}", tag=f"q{p}") for p in range(4)]
    k_t = [P.tile([128, T], BF16, name=f"k{p}", tag=f"k{p}") for p in range(4)]
    vaug_t = [P.tile([128, NH, CH + 1], BF16, name=f"va{s}", tag=f"va{s}") for s in range(ST)]
    wqk_t = P.tile([128, CT, 2 * C], F32R, name="wqk_t")
    wv_t = P.tile([128, CT, C], F32R, name="wv_t")
    wp_t = P.tile([128, CT, C], F32R, name="wp_t")
    gsel_t = P.tile([128, CT, NG], F32R, name="gsel_t")
    gselt_t = P.tile([NG, C], F32R, name="gselt_t")
    ab_t = [P.tile([128, 2], F32, name=f"ab{i}", tag=f"ab{i}") for i in range(CT)]
    bias_sb = P.tile([128, 20], F32, name="bias_sb")
    bqk_sb = bias_sb[:, 0:8]
    bproj_sb = bias_sb[:, 8:12]
    gamma_sb = bias_sb[:, 12:16]
    beta_sb = bias_sb[:, 16:20]
    ones_t = P.tile([128, 128], F32R, name="ones_t")
    eps_sb = P.tile([NG, 1], F32, name="eps_sb")
    mvr_sb = P.tile([NG, 2], F32R, name="mvr_sb")

    # ---- input DMAs: x on sync, weights on the gpsimd SWDGE queues ----
    # (separate semaphore pool, so a big weight transfer never blocks an
    # x tile's HWDGE slot)
    x_dmas = []
    for i in range(CT):
        for half in range(2):
            fs = slice(half * 512, half * 512 + 512)
            x_dmas.append(nc.sync.dma_start(
                out=x_t[i][:, fs], in_=x_d[i * 128:(i + 1) * 128, fs]))
    nc.gpsimd.dma_start(out=gsel_t, in_=gsel_d[:, :, :])
    nc.gpsimd.dma_start(out=gselt_t, in_=gselt_d[:, :])
    nc.gpsimd.dma_start(out=bias_sb, in_=bias_d[:, :])
    # weight transfers start only after x has landed: stats (and the whole
    # pipeline behind them) need x first, and HBM bandwidth is the startup
    # bottleneck. wqk is split so pair 0's q/k columns land first.
    w_dmas = [
        nc.gpsimd.dma_start(out=wqk_t[:, :, 0:256], in_=wqk_d[:, :, 0:256]),
        nc.gpsimd.dma_start(out=wqk_t[:, :, 256:2 * C], in_=wqk_d[:, :, 256:2 * C]),
        nc.gpsimd.dma_start(out=wv_t, in_=wv_d[:, :, :]),
        nc.gpsimd.dma_start(out=wp_t, in_=wproj_d[:, :, :]),
    ]
    for w in w_dmas:
        bass._add_dep_helper(w.ins, x_dmas[-1].ins, sync=True,
                             reason="weights yield HBM bandwidth to x")
    nc.gpsimd.memset(ones_t.bitcast(F32), 1.0)
    nc.vector.memset(eps_sb, EPS)

    # ---- phase 1: group norm statistics ----
    # per-channel sum via DVE tensor_scalar accumulate (dumps into xn, which
    # is fully overwritten later) and sum(x^2) via ACT Square accumulate
    # (dumps into h_t, dead until attention); both run while x halves land
    mv32_ps = PS.tile([NG, 2], F32, name="mv32_ps", tag="sc")
    for i in range(CT):
        st2 = W.tile([128, 2], F32R, name="st2", tag="st2", bufs=4)
        with nc.allow_low_precision(reason="f32r accum tags; same 4B payload"):
            nc.vector.tensor_scalar(out=xn_t[i], in0=x_t[i],
                                    scalar1=1.0, scalar2=0.0,
                                    op0=mybir.AluOpType.mult,
                                    op1=mybir.AluOpType.add,
                                    accum_out=st2[:, 0:1])
            nc.scalar.activation(out=h_t[i], in_=x_t[i],
                                 func=mybir.ActivationFunctionType.Square,
                                 accum_out=st2[:, 1:2])
        # group aggregation: (1/(16*1024)) * indicator^T @ [sum, sumsq]
        nc.tensor.matmul(out=mv32_ps, lhsT=gsel_t[:, i, :], rhs=st2,
                         start=(i == 0), stop=(i == CT - 1))
    mv32_sb = W.tile([NG, 2], F32, name="mv32_sb", tag="mv32", bufs=1)
    sdtmp = W.tile([NG, 2], F32, name="sdtmp", tag="sdtmp", bufs=1)
    nc.vector.tensor_copy(out=mv32_sb, in_=mv32_ps)
    # var_g = E[x^2]_g - mean_g^2 ; rstd = 1/sqrt(var+eps)
    nc.vector.tensor_mul(out=sdtmp[:, 0:1], in0=mv32_sb[:, 0:1], in1=mv32_sb[:, 0:1])
    nc.vector.tensor_sub(out=sdtmp[:, 1:2], in0=mv32_sb[:, 1:2], in1=sdtmp[:, 0:1])
    nc.scalar.activation(out=sdtmp[:, 1:2], in_=sdtmp[:, 1:2],
                         func=mybir.ActivationFunctionType.Sqrt,
                         bias=eps_sb, scale=1.0)
    with nc.allow_low_precision(reason="f32r tag on rstd; same 4-byte payload"):
        nc.vector.reciprocal(out=mvr_sb[:, 1:2], in_=sdtmp[:, 1:2])
    nc.vector.tensor_copy(out=mvr_sb[:, 0:1], in_=mv32_sb[:, 0:1])
    # broadcast group stats back to channels, per-channel A/B, xn = x*A + B
    for i in range(CT):
        mr_ps = PS.tile([128, 2], F32, name="mr_ps", tag="sc")
        nc.tensor.matmul(out=mr_ps, lhsT=gselt_t[:, i * 128:(i + 1) * 128],
                         rhs=mvr_sb, start=True, stop=True)
        abm = W.tile([128, 1], F32, name="abm", tag="abm", bufs=4)
        nc.vector.tensor_mul(out=ab_t[i][:, 0:1], in0=mr_ps[:, 1:2], in1=gamma_sb[:, i:i + 1])
        nc.vector.tensor_mul(out=abm, in0=mr_ps[:, 0:1], in1=ab_t[i][:, 0:1])
        nc.vector.tensor_sub(out=ab_t[i][:, 1:2], in0=beta_sb[:, i:i + 1], in1=abm)
        nc.vector.tensor_scalar(out=xn_t[i], in0=x_t[i],
                                scalar1=ab_t[i][:, 0:1], scalar2=ab_t[i][:, 1:2],
                                op0=mybir.AluOpType.mult, op1=mybir.AluOpType.add)

    # ---- deferred PE work: 4-matmul units spread across pairs 0..2 ----
    def qk_unit(oc, t2):
        p, is_k = oc // 2, oc % 2
        fs = slice(t2 * 512, t2 * 512 + 512)
        ps = PS.tile([128, 512], F32, name="qkps", tag="sc")
        for kk in range(CT):
            nc.tensor.matmul(out=ps,
                             lhsT=wqk_t[:, kk, oc * 128:(oc + 1) * 128],
                             rhs=xn_t[kk][:, fs],
                             start=(kk == 0), stop=(kk == CT - 1))
        dst = (k_t[p] if is_k else q_t[p])[:, fs]
        if use_bqk:
            nc.vector.tensor_scalar(out=dst, in0=ps, scalar1=bqk_sb[:, oc:oc + 1],
                                    scalar2=None, op0=mybir.AluOpType.add)
        else:
            nc.vector.tensor_copy(out=dst, in_=ps)

    def vt_unit(s):
        # whole-tile memset (strided ones-column memset fails this walrus);
        # the copy below overwrites cols 0..63 per head, col 64 stays 1.0
        nc.vector.memset(vaug_t[s], 1.0)
        ps = PS.tile([128, C], F32, name="vtps", tag="sc")
        for kk in range(CT):
            nc.tensor.matmul(out=ps,
                             lhsT=xn_t[kk][:, s * 128:(s + 1) * 128],
                             rhs=wv_t[:, kk, :],
                             start=(kk == 0), stop=(kk == CT - 1))
        nc.vector.tensor_copy(out=vaug_t[s][:, :, 0:CH],
                              in_=ps.rearrange("p (h e) -> p h e", e=CH))

    for oc in range(2):
        for t2 in range(TC):
            qk_unit(oc, t2)
    # (pair, ss) -> work units; vT for s must land by the position where
    # wv consumes vaug[s] (DELAY steps after scores s, counting the spill
    # into the next pair); qk for pair p anywhere before pair p
    sched = {}
    for s in range(6):
        sched.setdefault((0, s), []).append(lambda s=s: vt_unit(s))
    sched.setdefault((1, 0), []).append(lambda: vt_unit(6))
    sched.setdefault((1, 1), []).append(lambda: vt_unit(7))
    for j, (oc, t2) in enumerate([(oc, t2) for oc in (2, 3) for t2 in range(TC)]):
        sched.setdefault((0, 6 + j // 2), []).append(lambda oc=oc, t2=t2: qk_unit(oc, t2))
    for j, (oc, t2) in enumerate([(oc, t2) for oc in (4, 5) for t2 in range(TC)]):
        sched.setdefault((1, 2 + j), []).append(lambda oc=oc, t2=t2: qk_unit(oc, t2))
    for j, (oc, t2) in enumerate([(oc, t2) for oc in (6, 7) for t2 in range(TC)]):
        sched.setdefault((2, j), []).append(lambda oc=oc, t2=t2: qk_unit(oc, t2))

    # ---- attention: head pairs, wv DELAY steps behind scores/exp ----
    def division(p, hu_ps, htmp):
        """h = hu/D for pair p, into h_t[p].

        1/D on DVE costs 8 cycles/element of FREE dim, so the four D rows
        (4 x 512) are DMA-reshaped to (64,32) first: one 0.4us reciprocal.
        The reciprocal then takes a DRAM round trip so a partition-broadcast
        DMA (stride-0 partition reads are DRAM-only) can replicate each
        512-vector across 64 partitions for the final psum*sbuf multiply.
        Nothing lands on the scalar engine (pacing exp) or PE.
        """
        chains = [(half, t2) for half in range(2) for t2 in range(TC)]
        scr = DR.tile([4, 512], F32, name="scr", tag="scr", bufs=2)
        scr2 = DR.tile([4, 512], F32, name="scr2", tag="scr2", bufs=2)
        dall = W.tile([64, 32], F32, name="dall", tag="dall", bufs=2)
        ralt = W.tile([64, 32], F32, name="ralt", tag="ralt", bufs=2)
        for j, (half, t2) in enumerate(chains):
            dsb = W.tile([128, 512], F32, name="dsb", tag="dsb", bufs=4)
            nc.vector.tensor_copy(out=dsb[CH:CH + 1, :],
                                  in_=hu_ps[half][t2][CH:CH + 1, :])
            nc.sync.dma_start(out=scr[j:j + 1, :], in_=dsb[CH:CH + 1, :])
        nc.sync.dma_start(out=dall,
                          in_=scr.rearrange("a (b c) -> (a b) c", c=32))
        nc.vector.reciprocal(out=ralt, in_=dall)
        nc.sync.dma_start(out=scr2.rearrange("a (b c) -> (a b) c", c=32),
                          in_=ralt)
        for j, (half, t2) in enumerate(chains):
            fs = slice(t2 * 512, t2 * 512 + 512)
            rcb = W.tile([64, 512], F32, name="rcb", tag="rcb", bufs=4)
            row = scr2[j:j + 1, :]
            bcast = bass.AP(tensor=row.tensor, offset=row.offset,
                            ap=[[0, 64], list(row.ap[-1])])
            nc.sync.dma_start(out=rcb, in_=bcast)
            dst = h_t[p] if half == 0 else htmp
            with nc.allow_low_precision(reason="f32r tag on h; same payload"):
                nc.vector.tensor_mul(out=dst[0:CH, fs],
                                     in0=hu_ps[half][t2][0:CH, :],
                                     in1=rcb)
        # odd head's h goes to partitions 64..127 (cross-partition -> DMA)
        nc.sync.dma_start(out=h_t[p][64:128, :], in_=htmp[0:64, :])

    def division_act(p, hu_ps, htmp):
        """Tail variant: 1/D = exp(-ln D) on the (by now idle) scalar engine
        after a K=1 ones-matmul broadcasts D across partitions - avoids the
        ~10us of DMA round-trip latency the DMA-based division would expose
        at the end of the kernel."""
        for half in range(2):
            for t2 in range(TC):
                fs = slice(t2 * 512, t2 * 512 + 512)
                dsb = W.tile([128, 512], F32R, name="dsbt", tag="dsb", bufs=4)
                nc.vector.tensor_copy(out=dsb[CH:CH + 1, :],
                                      in_=hu_ps[half][t2][CH:CH + 1, :])
                bc = PS.tile([128, 512], F32, name="bcps", tag="sc")
                nc.tensor.matmul(out=bc, lhsT=ones_t[CH:CH + 1, :],
                                 rhs=dsb[CH:CH + 1, :], start=True, stop=True)
                lnd = W.tile([128, 512], F32, name="lnd", tag="lnd", bufs=2)
                nc.scalar.activation(out=lnd, in_=bc,
                                     func=mybir.ActivationFunctionType.Ln)
                rc = W.tile([128, 512], F32, name="rct", tag="rct", bufs=2)
                nc.scalar.activation(out=rc, in_=lnd,
                                     func=mybir.ActivationFunctionType.Exp,
                                     scale=-1.0)
                dst = h_t[p] if half == 0 else htmp
                with nc.allow_low_precision(reason="f32r tag on h; same payload"):
                    nc.vector.tensor_mul(out=dst[0:CH, fs],
                                         in0=hu_ps[half][t2][0:CH, :],
                                         in1=rc[0:CH, :])
        nc.sync.dma_start(out=h_t[p][64:128, :], in_=htmp[0:64, :])

    pend = None  # (p, hu_ps, htmp) awaiting division
    for p in range(4):
        hu_ps = [[None] * TC for _ in range(2)]
        htmp = W.tile([64, T], F32R, name="htmp", tag="htmp", bufs=2)
        expw = {}
        prev = pend
        for ss in range(ST):
            scps = [None, None]
            for half in range(2):
                base = 64 * half
                sc = PS.tile([128, T], F32, name=f"scps{half}", tag="sc")
                scps[half] = sc
                for t2 in range(TC):
                    fs = slice(t2 * 512, t2 * 512 + 512)
                    nc.tensor.matmul(
                        out=sc[:, fs],
                        lhsT=k_t[p][base:base + 64, ss * 128:(ss + 1) * 128],
                        rhs=q_t[p][base:base + 64, fs],
                        start=True, stop=True)
                ew = W.tile([128, T], BF16, name="expw", tag="expw", bufs=12)
                nc.scalar.activation(out=ew, in_=scps[half],
                                     func=mybir.ActivationFunctionType.Exp,
                                     scale=0.125)
                expw[half, ss] = ew
            for unit in sched.get((p, ss), []):
                unit()
            if prev is not None:
                # previous pair's wv tail (ss 5..7) and then its division
                if ss < DELAY:
                    _wv(nc, PS, vaug_t, prev[3], prev[1], prev[0], ST - DELAY + ss)
                elif ss == DELAY:
                    division(prev[0], prev[1], prev[2])
                    prev = pend = None
            if ss >= DELAY:
                _wv(nc, PS, vaug_t, expw, hu_ps, p, ss - DELAY)
        pend = (p, hu_ps, htmp, expw)
    for ss in range(ST - DELAY, ST):
        _wv(nc, PS, vaug_t, pend[3], pend[1], pend[0], ss)
    division_act(pend[0], pend[1], pend[2])

    # ---- proj + residual ----
    for oc in range(CT):
        ps = PS.tile([128, T], F32, name="prps", tag="sc")
        for t2 in range(TC):
            fs = slice(t2 * 512, t2 * 512 + 512)
            for kk in range(CT):
                nc.tensor.matmul(out=ps[:, fs],
                                 lhsT=wp_t[:, kk, oc * 128:(oc + 1) * 128],
                                 rhs=h_t[kk][:, fs],
                                 start=(kk == 0), stop=(kk == CT - 1))
        if use_bproj:
            prtmp = W.tile([128, T], F32, name="prtmp", tag="prtmp", bufs=2)
            nc.vector.tensor_scalar(out=prtmp, in0=ps, scalar1=bproj_sb[:, oc:oc + 1],
                                    scalar2=None, op0=mybir.AluOpType.add)
            with nc.allow_low_precision(reason="f32r tag on out; same payload"):
                nc.vector.tensor_add(out=x_t[oc], in0=x_t[oc], in1=prtmp)
        else:
            with nc.allow_low_precision(reason="f32r tag on out; same payload"):
                nc.vector.tensor_add(out=x_t[oc], in0=x_t[oc], in1=ps)
        nc.sync.dma_start(out=out_d[oc * 128:(oc + 1) * 128, :], in_=x_t[oc])


def _wv(nc, PS, vaug_t, expw, hu_ps, p, ss):
    for half in range(2):
        h = 2 * p + half
        for t2 in range(TC):
            if ss == 0:
                hu_ps[half][t2] = PS.tile([128, 512], F32,
                                          name="hups", tag="hu", bufs=4)
            fs = slice(t2 * 512, t2 * 512 + 512)
            nc.tensor.matmul(out=hu_ps[half][t2][0:CH + 1, :],
                             lhsT=vaug_t[ss][:, h, :],
                             rhs=expw[half, ss][:, fs],
                             start=(ss == 0), stop=(ss == ST - 1))


_PROGRAM_CACHE = {}


def _get_program(use_bqk, use_bproj):
    key = (use_bqk, use_bproj)
    if key not in _PROGRAM_CACHE:
        _PROGRAM_CACHE[key] = build_program(*key)
    return _PROGRAM_CACHE[key]


def make_host_inputs(x, gamma, beta, w_qkv, b_qkv, w_proj, b_proj):
    """Host-side preprocessing shared by all cores."""
    x = np.asarray(x, np.float32)
    w_qkv = np.asarray(w_qkv, np.float32)
    b_qkv = np.asarray(b_qkv, np.float32)
    w_proj = np.asarray(w_proj, np.float32)
    b_proj = np.asarray(b_proj, np.float32)
    gamma = np.asarray(gamma, np.float32)
    beta = np.asarray(beta, np.float32)

    # per-head slices of w_qkv rows (3c, c): head h -> q,k,v at 192h+{0,64,128}
    wq = np.stack([w_qkv[192 * h:192 * h + 64] for h in range(NH)])
    wk = np.stack([w_qkv[192 * h + 64:192 * h + 128] for h in range(NH)])
    wv = np.stack([w_qkv[192 * h + 128:192 * h + 192] for h in range(NH)])
    bq = np.stack([b_qkv[192 * h:192 * h + 64] for h in range(NH)])
    bk = np.stack([b_qkv[192 * h + 64:192 * h + 128] for h in range(NH)])
    bv = np.stack([b_qkv[192 * h + 128:192 * h + 192] for h in range(NH)])

    # wqk (512c, 1024): chunk 2p = q of heads (2p,2p+1), chunk 2p+1 = k of same
    chunks, bqk_chunks = [], []
    for p in range(4):
        chunks.append(np.concatenate([wq[2 * p], wq[2 * p + 1]], 0).T)
        chunks.append(np.concatenate([wk[2 * p], wk[2 * p + 1]], 0).T)
        bqk_chunks.append(np.concatenate([bq[2 * p], bq[2 * p + 1]], 0))
        bqk_chunks.append(np.concatenate([bk[2 * p], bk[2 * p + 1]], 0))
    wqk_host = np.concatenate(chunks, axis=1)                     # (512,1024)
    bqk_host = np.stack(bqk_chunks, axis=1)                       # (128,8)

    wvT_host = wv.reshape(C, C).T.copy()                          # (512c, 512vch)
    wprojT_host = w_proj.T.copy()                                 # (512c, 512o)
    # v-bias contributes exactly b_v through the softmax (rows sum to 1);
    # fold it into the proj bias
    bproj_eff = b_proj + w_proj @ bv.reshape(C)
    bproj_host = bproj_eff.reshape(CT, 128).T.copy()
    gamma_host = gamma.reshape(CT, 128).T.copy()
    beta_host = beta.reshape(CT, 128).T.copy()

    cidx = np.arange(C)
    # gsel aggregates raw [sum, sum(x^2)] rows -> per-group means
    gsel_host = (cidx[:, None] // GS == np.arange(NG)[None, :]).astype(np.float32) / (GS * T)
    gselt_host = (cidx[None, :] // GS == np.arange(NG)[:, None]).astype(np.float32)

    def ktile(a):
        # (512, N) -> (128, 4, N): partition-major layout for one big tile
        return np.ascontiguousarray(a.reshape(CT, 128, -1).transpose(1, 0, 2))

    use_bqk = bool(np.any(bqk_host))
    use_bproj = bool(np.any(bproj_host))
    biases = np.concatenate([bqk_host, bproj_host, gamma_host, beta_host], axis=1)

    common = {
        "wqk": ktile(wqk_host),
        "wv": ktile(wvT_host),
        "wproj": ktile(wprojT_host),
        "gsel": ktile(gsel_host),
        "gselt": np.ascontiguousarray(gselt_host),
        "biases": np.ascontiguousarray(biases.astype(np.float32)),
    }
    return x, common, use_bqk, use_bproj


def kernel(x, gamma, beta, w_qkv, b_qkv, w_proj, b_proj):
    b, c, H, Wd = x.shape
    assert (b, c, H, Wd) == (8, C, 32, 32)
    xf, common, use_bqk, use_bproj = make_host_inputs(
        x, gamma, beta, w_qkv, b_qkv, w_proj, b_proj)
    xf = xf.reshape(b, C, T)

    nc = _get_program(use_bqk, use_bproj)
    if not getattr(nc, "_waits_split", False):
        _split_multi_waits(nc)
        nc._waits_split = True
    in_maps = [dict(common, x=np.ascontiguousarray(xf[i])) for i in range(NCORES)]
    res = run_bass_kernel_spmd(nc, in_maps, list(range(NCORES)))
    out = np.stack([res.results[i]["out"] for i in range(NCORES)])
    return out.reshape(b, C, H, Wd).astype(np.float32)


if __name__ == "__main__":
    rng = np.random.default_rng(0)
    args = {
        "x": rng.standard_normal((8, C, 32, 32), dtype=np.float32),
        "gamma": np.ones(C, np.float32),
        "beta": np.zeros(C, np.float32),
        "w_qkv": (rng.standard_normal((3 * C, C)) * 0.02).astype(np.float32),
        "b_qkv": np.zeros(3 * C, np.float32),
        "w_proj": (rng.standard_normal((C, C)) * 0.02).astype(np.float32),
        "b_proj": np.zeros(C, np.float32),
    }
    out = kernel(**args)
    print(out.shape, out.dtype)


# revision 28
# speedup vs baseline: 1.2051x; 1.1082x over previous
"""AttentionBlock kernel for 8 Trainium2 NeuronCores.

Problem: x(8,512,32,32) -> GroupNorm(32) -> qkv 1x1 conv -> 8-head attention
         over T=1024 tokens -> proj 1x1 conv -> residual.

Sharding: pure data parallel - one batch element per core, no collectives.

Per-core dataflow (c=512 channels on partitions, T=1024 tokens on free dim):
  1. GroupNorm stats: bn_stats per channel; tiny PE matmuls (group-selector
     matrices) aggregate across the 16 channels of each group and broadcast
     group stats back to channels; xn = x*A + B via one tensor_scalar per tile.
  2. q,k projection (fp32r) with head-pair-permuted weights; outputs cast to
     bf16: q/k of heads (2p,2p+1) stacked on partitions 0-63/64-127 of one
     (128,1024) tile each, so the K=64 score matmuls of a pair land in
     disjoint PE row groups.
  3. v is produced TRANSPOSED directly by the matmul vT = xn^T @ wvT
     (lhsT = xn), avoiding any explicit transpose for the second attention
     matmul; cast to bf16 with an all-ones column appended (M=65) so the WV
     matmul also emits the softmax denominator D[t] as output row 64.
  4. scores^T(s,t) = k^T q per head in bf16 (hw runs K<128 fp32r matmuls at
     half rate; bf16 runs at full rate and the ~4e-3 rounding is well within
     tolerance).
  5. exp on the scalar engine reading PSUM, writing bf16 (scale=0.125 folds
     the attention scaling exactly); WV accumulates hu (and D at row 64) in
     fp32 PSUM. 1/D via reciprocal_approx_fast after a K=1 ones-matmul
     broadcasts D to all partitions.
  6. proj (fp32r) + residual into the x tiles, DMA out.

Schedule: WV runs 3 pipeline steps behind scores/exp so the scalar engine
never starves; the previous pair's division chain is emitted inside the next
pair (keeps the in-order PE queue from stalling on it); qk chunks 2..7 and
all vT matmuls are spread across pair 0's pipeline steps.
"""

import sys

for _p in ("/opt/trn_rl_repo",):
    if _p not in sys.path:
        sys.path.insert(0, _p)

import numpy as np

import concourse.bass as bass
import concourse.tile as tile
from concourse import mybir
from concourse.bass_utils import run_bass_kernel_spmd
from concourse.vector_clock import ScopedClock, VectorClock


def _patched_drain_and_barrier(self, tick_clock, wait_clock):
    # This container's walrus rejects instructions carrying more than one
    # sync wait. Split the final drain's global-clock waits across NOPs.
    g = tick_clock.global_clock
    n = len(g)
    for lo in range(0, n, 4):
        vec = [g[p] if lo <= p < lo + 4 else 0 for p in range(n)]
        if not any(vec):
            continue
        nop_inst = self.nc.sync.nop()
        wait_clock.add_sem_waits(nop_inst.ins, ScopedClock({None: VectorClock(vec)}))
    self.nc.sync.drain()
    self.nc.all_engine_barrier()
    assert self.sems is not None
    popped = self.nc._tile_sem_poison_stack.pop()
    assert popped is self._sem_poison
    self.nc.clear_and_free_semaphores(list(self.sems.allocated().values()))
    self.nc.all_engine_barrier()


tile.TileContext._drain_and_barrier = _patched_drain_and_barrier


def _split_multi_waits(nc):
    # This walrus build accepts at most one sync wait per instruction. Hoist
    # surplus waits onto same-engine NOPs placed immediately before.
    n = 0
    for fn in nc.m.functions:
        for blk in fn.blocks:
            out = []
            for inst in blk.instructions:
                si = inst.sync_info
                if si is not None and si.on_wait and len(si.on_wait) > 1:
                    waits = list(si.on_wait)
                    for w in waits[:-1]:
                        nop = mybir.InstNoOp(name=f"{inst.name}_w{n}", ins=[], outs=[])
                        n += 1
                        nop.engine = inst.engine
                        nop.sync_info = mybir.SyncInfo(on_wait=[w], on_update=[])
                        out.append(nop)
                    si.on_wait = [waits[-1]]
                out.append(inst)
            blk.instructions = out


F32 = mybir.dt.float32
F32R = mybir.dt.float32r
BF16 = mybir.dt.bfloat16

C = 512
T = 1024
NH = 8          # heads
CH = C // NH    # 64 channels per head
NG = 32         # groups
GS = C // NG    # 16 channels per group
EPS = 1e-5
NCORES = 8
CT = C // 128   # 4 channel tiles
ST = T // 128   # 8 s tiles
TC = T // 512   # 2 t chunks
DELAY = 4       # wv runs this many pipeline steps behind scores/exp


def build_program(use_bqk: bool, use_bproj: bool) -> bass.Bass:
    nc = bass.Bass()

    x_d = nc.dram_tensor("x", [C, T], F32R, kind="ExternalInput")
    wqk_d = nc.dram_tensor("wqk", [128, CT, 2 * C], F32R, kind="ExternalInput")
    wv_d = nc.dram_tensor("wv", [128, CT, C], F32R, kind="ExternalInput")
    wproj_d = nc.dram_tensor("wproj", [128, CT, C], F32R, kind="ExternalInput")
    gsel_d = nc.dram_tensor("gsel", [128, CT, NG], F32R, kind="ExternalInput")
    gselt_d = nc.dram_tensor("gselt", [NG, C], F32R, kind="ExternalInput")
    # combined small params: [bqk(8) | bproj(4) | gamma(4) | beta(4)]
    bias_d = nc.dram_tensor("biases", [128, 20], F32, kind="ExternalInput")
    out_d = nc.dram_tensor("out", [C, T], F32R, kind="ExternalOutput")

    with tile.TileContext(nc) as tc:
        with (
            tc.tile_pool(name="persist", bufs=1) as P,
            tc.tile_pool(name="work", bufs=2) as W,
            tc.tile_pool(name="ps", bufs=2, space="PSUM") as PS,
            tc.tile_pool(name="dr", bufs=2, space="DRAM") as DR,
        ):
            _body(nc, tc, P, W, PS, DR, locals(), use_bqk, use_bproj)
    return nc


def _body(nc, tc, P, W, PS, DR, d, use_bqk, use_bproj):
    x_d, wqk_d, wv_d, wproj_d = d["x_d"], d["wqk_d"], d["wv_d"], d["wproj_d"]
    gsel_d, gselt_d, bias_d, out_d = d["gsel_d"], d["gselt_d"], d["bias_d"], d["out_d"]

    # ---- persistent SBUF tiles ----
    x_t = [P.tile([128, T], F32R, name=f"x{i}", tag=f"x{i}") for i in range(CT)]
    xn_t = [P.tile([128, T], F32R, name=f"xn{i}", tag=f"xn{i}") for i in range(CT)]
    h_t = [P.tile([128, T], F32R, name=f"h{i}", tag=f"h{i}") for i in range(CT)]
    q_t = [P.tile([128, T], BF16, name=f"q{p}", tag=f"q{p}") for p in range(4)]
    k_t = [P.tile([128, T], BF16, name=f"k{p}", tag=f"k{p}") for p in range(4)]
    vaug_t = [P.tile([128, NH, CH + 1], BF16, name=f"va{s}", tag=f"va{s}") for s in range(ST)]
    wqk_t = P.tile([128, CT, 2 * C], F32R, name="wqk_t")
    wv_t = P.tile([128, CT, C], F32R, name="wv_t")
    wp_t = P.tile([128, CT, C], F32R, name="wp_t")
    gsel_t = P.tile([128, CT, NG], F32R, name="gsel_t")
    gselt_t = P.tile([NG, C], F32R, name="gselt_t")
    ab_t = [P.tile([128, 2], F32, name=f"ab{i}", tag=f"ab{i}") for i in range(CT)]
    bias_sb = P.tile([128, 20], F32, name="bias_sb")
    bqk_sb = bias_sb[:, 0:8]
    bproj_sb = bias_sb[:, 8:12]
    gamma_sb = bias_sb[:, 12:16]
    beta_sb = bias_sb[:, 16:20]
    ones_t = P.tile([128, 128], F32R, name="ones_t")
    eps_sb = P.tile([NG, 1], F32, name="eps_sb")
    mvr_sb = P.tile([NG, 2], F32R, name="mvr_sb")

    # ---- input DMAs: x on sync, weights on the gpsimd SWDGE queues ----
    # (separate semaphore pool, so a big weight transfer never blocks an
    # x tile's HWDGE slot)
    x_dmas = []
    for i in range(CT):
        for half in range(2):
            fs = slice(half * 512, half * 512 + 512)
            x_dmas.append(nc.sync.dma_start(
                out=x_t[i][:, fs], in_=x_d[i * 128:(i + 1) * 128, fs]))
    nc.gpsimd.dma_start(out=gsel_t, in_=gsel_d[:, :, :])
    nc.gpsimd.dma_start(out=gselt_t, in_=gselt_d[:, :])
    nc.gpsimd.dma_start(out=bias_sb, in_=bias_d[:, :])
    # weight transfers start only after x has landed: stats (and the whole
    # pipeline behind them) need x first, and HBM bandwidth is the startup
    # bottleneck. wqk is split so pair 0's q/k columns land first.
    w_dmas = [
        nc.gpsimd.dma_start(out=wqk_t[:, :, 0:256], in_=wqk_d[:, :, 0:256]),
        nc.gpsimd.dma_start(out=wqk_t[:, :, 256:2 * C], in_=wqk_d[:, :, 256:2 * C]),
        nc.gpsimd.dma_start(out=wv_t, in_=wv_d[:, :, :]),
        nc.gpsimd.dma_start(out=wp_t, in_=wproj_d[:, :, :]),
    ]
    for w in w_dmas:
        bass._add_dep_helper(w.ins, x_dmas[-1].ins, sync=True,
                             reason="weights yield HBM bandwidth to x")
    nc.gpsimd.memset(ones_t.bitcast(F32), 1.0)
    nc.vector.memset(eps_sb, EPS)

    # ---- phase 1: group norm statistics ----
    # per-channel sum via DVE tensor_scalar accumulate (dumps into xn, which
    # is fully overwritten later) and sum(x^2) via ACT Square accumulate
    # (dumps into h_t, dead until attention); both run while x halves land
    mv32_ps = PS.tile([NG, 2], F32, name="mv32_ps", tag="sc")
    for i in range(CT):
        st2 = W.tile([128, 2], F32R, name="st2", tag="st2", bufs=4)
        with nc.allow_low_precision(reason="f32r accum tags; same 4B payload"):
            nc.vector.tensor_scalar(out=xn_t[i], in0=x_t[i],
                                    scalar1=1.0, scalar2=0.0,
                                    op0=mybir.AluOpType.mult,
                                    op1=mybir.AluOpType.add,
                                    accum_out=st2[:, 0:1])
            nc.scalar.activation(out=h_t[i], in_=x_t[i],
                                 func=mybir.ActivationFunctionType.Square,
                                 accum_out=st2[:, 1:2])
        # group aggregation: (1/(16*1024)) * indicator^T @ [sum, sumsq]
        nc.tensor.matmul(out=mv32_ps, lhsT=gsel_t[:, i, :], rhs=st2,
                         start=(i == 0), stop=(i == CT - 1))
    mv32_sb = W.tile([NG, 2], F32, name="mv32_sb", tag="mv32", bufs=1)
    sdtmp = W.tile([NG, 2], F32, name="sdtmp", tag="sdtmp", bufs=1)
    nc.vector.tensor_copy(out=mv32_sb, in_=mv32_ps)
    # var_g = E[x^2]_g - mean_g^2 ; rstd = 1/sqrt(var+eps)
    nc.vector.tensor_mul(out=sdtmp[:, 0:1], in0=mv32_sb[:, 0:1], in1=mv32_sb[:, 0:1])
    nc.vector.tensor_sub(out=sdtmp[:, 1:2], in0=mv32_sb[:, 1:2], in1=sdtmp[:, 0:1])
    nc.scalar.activation(out=sdtmp[:, 1:2], in_=sdtmp[:, 1:2],
                         func=mybir.ActivationFunctionType.Sqrt,
                         bias=eps_sb, scale=1.0)
    with nc.allow_low_precision(reason="f32r tag on rstd; same 4-byte payload"):
        nc.vector.reciprocal(out=mvr_sb[:, 1:2], in_=sdtmp[:, 1:2])
    nc.vector.tensor_copy(out=mvr_sb[:, 0:1], in_=mv32_sb[:, 0:1])
    # broadcast group stats back to channels, per-channel A/B, xn = x*A + B
    for i in range(CT):
        mr_ps = PS.tile([128, 2], F32, name="mr_ps", tag="sc")
        nc.tensor.matmul(out=mr_ps, lhsT=gselt_t[:, i * 128:(i + 1) * 128],
                         rhs=mvr_sb, start=True, stop=True)
        abm = W.tile([128, 1], F32, name="abm", tag="abm", bufs=4)
        nc.vector.tensor_mul(out=ab_t[i][:, 0:1], in0=mr_ps[:, 1:2], in1=gamma_sb[:, i:i + 1])
        nc.vector.tensor_mul(out=abm, in0=mr_ps[:, 0:1], in1=ab_t[i][:, 0:1])
        nc.vector.tensor_sub(out=ab_t[i][:, 1:2], in0=beta_sb[:, i:i + 1], in1=abm)
        nc.vector.tensor_scalar(out=xn_t[i], in0=x_t[i],
                                scalar1=ab_t[i][:, 0:1], scalar2=ab_t[i][:, 1:2],
                                op0=mybir.AluOpType.mult, op1=mybir.AluOpType.add)

    # ---- deferred PE work: 4-matmul units spread across pairs 0..2 ----
    def qk_unit(oc, t2):
        p, is_k = oc // 2, oc % 2
        fs = slice(t2 * 512, t2 * 512 + 512)
        ps = PS.tile([128, 512], F32, name="qkps", tag="sc")
        for kk in range(CT):
            nc.tensor.matmul(out=ps,
                             lhsT=wqk_t[:, kk, oc * 128:(oc + 1) * 128],
                             rhs=xn_t[kk][:, fs],
                             start=(kk == 0), stop=(kk == CT - 1))
        dst = (k_t[p] if is_k else q_t[p])[:, fs]
        if use_bqk:
            nc.vector.tensor_scalar(out=dst, in0=ps, scalar1=bqk_sb[:, oc:oc + 1],
                                    scalar2=None, op0=mybir.AluOpType.add)
        else:
            nc.vector.tensor_copy(out=dst, in_=ps)

    def vt_unit(s):
        # whole-tile memset (strided ones-column memset fails this walrus);
        # the copy below overwrites cols 0..63 per head, col 64 stays 1.0
        nc.vector.memset(vaug_t[s], 1.0)
        ps = PS.tile([128, C], F32, name="vtps", tag="sc")
        for kk in range(CT):
            nc.tensor.matmul(out=ps,
                             lhsT=xn_t[kk][:, s * 128:(s + 1) * 128],
                             rhs=wv_t[:, kk, :],
                             start=(kk == 0), stop=(kk == CT - 1))
        nc.vector.tensor_copy(out=vaug_t[s][:, :, 0:CH],
                              in_=ps.rearrange("p (h e) -> p h e", e=CH))

    for oc in range(2):
        for t2 in range(TC):
            qk_unit(oc, t2)
    # (pair, ss) -> work units; vT for s must land by the position where
    # wv consumes vaug[s] (DELAY steps after scores s, counting the spill
    # into the next pair); qk for pair p anywhere before pair p
    sched = {}
    for s in range(6):
        sched.setdefault((0, s), []).append(lambda s=s: vt_unit(s))
    sched.setdefault((1, 0), []).append(lambda: vt_unit(6))
    sched.setdefault((1, 1), []).append(lambda: vt_unit(7))
    for j, (oc, t2) in enumerate([(oc, t2) for oc in (2, 3) for t2 in range(TC)]):
        sched.setdefault((0, 6 + j // 2), []).append(lambda oc=oc, t2=t2: qk_unit(oc, t2))
    for j, (oc, t2) in enumerate([(oc, t2) for oc in (4, 5) for t2 in range(TC)]):
        sched.setdefault((1, 2 + j), []).append(lambda oc=oc, t2=t2: qk_unit(oc, t2))
    for j, (oc, t2) in enumerate([(oc, t2) for oc in (6, 7) for t2 in range(TC)]):
        sched.setdefault((2, j), []).append(lambda oc=oc, t2=t2: qk_unit(oc, t2))

    # ---- attention: head pairs, wv DELAY steps behind scores/exp ----
    def division(p, hu_ps, htmp):
        """h = hu/D for pair p, into h_t[p].

        1/D on DVE costs 8 cycles/element of FREE dim, so the four D rows
        (4 x 512) are DMA-reshaped to (64,32) first: one 0.4us reciprocal.
        The reciprocal then takes a DRAM round trip so a partition-broadcast
        DMA (stride-0 partition reads are DRAM-only) can replicate each
        512-vector across 64 partitions for the final psum*sbuf multiply.
        Nothing lands on the scalar engine (pacing exp) or PE.
        """
        chains = [(half, t2) for half in range(2) for t2 in range(TC)]
        scr = DR.tile([4, 512], F32, name="scr", tag="scr", bufs=2)
        scr2 = DR.tile([4, 512], F32, name="scr2", tag="scr2", bufs=2)
        dall = W.tile([64, 32], F32, name="dall", tag="dall", bufs=2)
        ralt = W.tile([64, 32], F32, name="ralt", tag="ralt", bufs=2)
        husb = []
        for j, (half, t2) in enumerate(chains):
            hs = W.tile([128, 512], F32, name="husb", tag="husb", bufs=8)
            husb.append(hs)
            # copy h rows + D row out of PSUM so the accumulate banks free
            # immediately instead of waiting out the division's DMA latency
            nc.vector.tensor_copy(out=hs[0:CH + 1, :],
                                  in_=hu_ps[half][t2][0:CH + 1, :])
            nc.sync.dma_start(out=scr[j:j + 1, :], in_=hs[CH:CH + 1, :])
        nc.sync.dma_start(out=dall,
                          in_=scr.rearrange("a (b c) -> (a b) c", c=32))
        nc.vector.reciprocal(out=ralt, in_=dall)
        nc.sync.dma_start(out=scr2.rearrange("a (b c) -> (a b) c", c=32),
                          in_=ralt)
        for j, (half, t2) in enumerate(chains):
            fs = slice(t2 * 512, t2 * 512 + 512)
            rcb = W.tile([64, 512], F32, name="rcb", tag="rcb", bufs=4)
            row = scr2[j:j + 1, :]
            bcast = bass.AP(tensor=row.tensor, offset=row.offset,
                            ap=[[0, 64], list(row.ap[-1])])
            nc.sync.dma_start(out=rcb, in_=bcast)
            dst = h_t[p] if half == 0 else htmp
            with nc.allow_low_precision(reason="f32r tag on h; same payload"):
                nc.vector.tensor_mul(out=dst[0:CH, fs],
                                     in0=husb[j][0:CH, :],
                                     in1=rcb)
        # odd head's h goes to partitions 64..127 (cross-partition -> DMA)
        nc.sync.dma_start(out=h_t[p][64:128, :], in_=htmp[0:64, :])

    def division_act(p, hu_ps, htmp):
        """Tail variant: 1/D = exp(-ln D) on the (by now idle) scalar engine
        after a K=1 ones-matmul broadcasts D across partitions - avoids the
        ~10us of DMA round-trip latency the DMA-based division would expose
        at the end of the kernel."""
        for half in range(2):
            for t2 in range(TC):
                fs = slice(t2 * 512, t2 * 512 + 512)
                dsb = W.tile([128, 512], F32R, name="dsbt", tag="dsb", bufs=4)
                nc.vector.tensor_copy(out=dsb[CH:CH + 1, :],
                                      in_=hu_ps[half][t2][CH:CH + 1, :])
                bc = PS.tile([128, 512], F32, name="bcps", tag="sc")
                nc.tensor.matmul(out=bc, lhsT=ones_t[CH:CH + 1, :],
                                 rhs=dsb[CH:CH + 1, :], start=True, stop=True)
                lnd = W.tile([128, 512], F32, name="lnd", tag="lnd", bufs=2)
                nc.scalar.activation(out=lnd, in_=bc,
                                     func=mybir.ActivationFunctionType.Ln)
                rc = W.tile([128, 512], F32, name="rct", tag="rct", bufs=2)
                nc.scalar.activation(out=rc, in_=lnd,
                                     func=mybir.ActivationFunctionType.Exp,
                                     scale=-1.0)
                dst = h_t[p] if half == 0 else htmp
                with nc.allow_low_precision(reason="f32r tag on h; same payload"):
                    nc.vector.tensor_mul(out=dst[0:CH, fs],
                                         in0=hu_ps[half][t2][0:CH, :],
                                         in1=rc[0:CH, :])
        nc.sync.dma_start(out=h_t[p][64:128, :], in_=htmp[0:64, :])

    pend = None  # (p, hu_ps, htmp) awaiting division
    for p in range(4):
        hu_ps = [[None] * TC for _ in range(2)]
        htmp = W.tile([64, T], F32R, name="htmp", tag="htmp", bufs=2)
        expw = {}
        prev = pend
        for ss in range(ST):
            scps = [None, None]
            for half in range(2):
                base = 64 * half
                sc = PS.tile([128, T], F32, name=f"scps{half}", tag="sc")
                scps[half] = sc
                for t2 in range(TC):
                    fs = slice(t2 * 512, t2 * 512 + 512)
                    nc.tensor.matmul(
                        out=sc[:, fs],
                        lhsT=k_t[p][base:base + 64, ss * 128:(ss + 1) * 128],
                        rhs=q_t[p][base:base + 64, fs],
                        start=True, stop=True)
                ew = W.tile([128, T], BF16, name="expw", tag="expw", bufs=14)
                nc.scalar.activation(out=ew, in_=scps[half],
                                     func=mybir.ActivationFunctionType.Exp,
                                     scale=0.125)
                expw[half, ss] = ew
            for unit in sched.get((p, ss), []):
                unit()
            if prev is not None:
                # previous pair's wv tail (ss ST-DELAY..ST-1), then division
                if ss < DELAY:
                    _wv(nc, PS, vaug_t, prev[3], prev[1], prev[0], ST - DELAY + ss)
                if ss == DELAY - 1:
                    division(prev[0], prev[1], prev[2])
                    prev = pend = None
            if ss >= DELAY:
                _wv(nc, PS, vaug_t, expw, hu_ps, p, ss - DELAY)
        pend = (p, hu_ps, htmp, expw)
    for ss in range(ST - DELAY, ST):
        _wv(nc, PS, vaug_t, pend[3], pend[1], pend[0], ss)
    division_act(pend[0], pend[1], pend[2])

    # ---- proj + residual ----
    for oc in range(CT):
        ps = PS.tile([128, T], F32, name="prps", tag="sc")
        for t2 in range(TC):
            fs = slice(t2 * 512, t2 * 512 + 512)
            for kk in range(CT):
                nc.tensor.matmul(out=ps[:, fs],
                                 lhsT=wp_t[:, kk, oc * 128:(oc + 1) * 128],
                                 rhs=h_t[kk][:, fs],
                                 start=(kk == 0), stop=(kk == CT - 1))
        if use_bproj:
            prtmp = W.tile([128, T], F32, name="prtmp", tag="prtmp", bufs=2)
            nc.vector.tensor_scalar(out=prtmp, in0=ps, scalar1=bproj_sb[:, oc:oc + 1],
                                    scalar2=None, op0=mybir.AluOpType.add)
            with nc.allow_low_precision(reason="f32r tag on out; same payload"):
                nc.vector.tensor_add(out=x_t[oc], in0=x_t[oc], in1=prtmp)
        else:
            with nc.allow_low_precision(reason="f32r tag on out; same payload"):
                nc.vector.tensor_add(out=x_t[oc], in0=x_t[oc], in1=ps)
        nc.sync.dma_start(out=out_d[oc * 128:(oc + 1) * 128, :], in_=x_t[oc])


def _wv(nc, PS, vaug_t, expw, hu_ps, p, ss):
    for half in range(2):
        h = 2 * p + half
        for t2 in range(TC):
            if ss == 0:
                hu_ps[half][t2] = PS.tile([128, 512], F32,
                                          name="hups", tag="hu", bufs=4)
            fs = slice(t2 * 512, t2 * 512 + 512)
            nc.tensor.matmul(out=hu_ps[half][t2][0:CH + 1, :],
                             lhsT=vaug_t[ss][:, h, :],
                             rhs=expw[half, ss][:, fs],
                             start=(ss == 0), stop=(ss == ST - 1))


_PROGRAM_CACHE = {}


def _get_program(use_bqk, use_bproj):
    key = (use_bqk, use_bproj)
    if key not in _PROGRAM_CACHE:
        _PROGRAM_CACHE[key] = build_program(*key)
    return _PROGRAM_CACHE[key]


def make_host_inputs(x, gamma, beta, w_qkv, b_qkv, w_proj, b_proj):
    """Host-side preprocessing shared by all cores."""
    x = np.asarray(x, np.float32)
    w_qkv = np.asarray(w_qkv, np.float32)
    b_qkv = np.asarray(b_qkv, np.float32)
    w_proj = np.asarray(w_proj, np.float32)
    b_proj = np.asarray(b_proj, np.float32)
    gamma = np.asarray(gamma, np.float32)
    beta = np.asarray(beta, np.float32)

    # per-head slices of w_qkv rows (3c, c): head h -> q,k,v at 192h+{0,64,128}
    wq = np.stack([w_qkv[192 * h:192 * h + 64] for h in range(NH)])
    wk = np.stack([w_qkv[192 * h + 64:192 * h + 128] for h in range(NH)])
    wv = np.stack([w_qkv[192 * h + 128:192 * h + 192] for h in range(NH)])
    bq = np.stack([b_qkv[192 * h:192 * h + 64] for h in range(NH)])
    bk = np.stack([b_qkv[192 * h + 64:192 * h + 128] for h in range(NH)])
    bv = np.stack([b_qkv[192 * h + 128:192 * h + 192] for h in range(NH)])

    # wqk (512c, 1024): chunk 2p = q of heads (2p,2p+1), chunk 2p+1 = k of same
    chunks, bqk_chunks = [], []
    for p in range(4):
        chunks.append(np.concatenate([wq[2 * p], wq[2 * p + 1]], 0).T)
        chunks.append(np.concatenate([wk[2 * p], wk[2 * p + 1]], 0).T)
        bqk_chunks.append(np.concatenate([bq[2 * p], bq[2 * p + 1]], 0))
        bqk_chunks.append(np.concatenate([bk[2 * p], bk[2 * p + 1]], 0))
    wqk_host = np.concatenate(chunks, axis=1)                     # (512,1024)
    bqk_host = np.stack(bqk_chunks, axis=1)                       # (128,8)

    wvT_host = wv.reshape(C, C).T.copy()                          # (512c, 512vch)
    wprojT_host = w_proj.T.copy()                                 # (512c, 512o)
    # v-bias contributes exactly b_v through the softmax (rows sum to 1);
    # fold it into the proj bias
    bproj_eff = b_proj + w_proj @ bv.reshape(C)
    bproj_host = bproj_eff.reshape(CT, 128).T.copy()
    gamma_host = gamma.reshape(CT, 128).T.copy()
    beta_host = beta.reshape(CT, 128).T.copy()

    cidx = np.arange(C)
    # gsel aggregates raw [sum, sum(x^2)] rows -> per-group means
    gsel_host = (cidx[:, None] // GS == np.arange(NG)[None, :]).astype(np.float32) / (GS * T)
    gselt_host = (cidx[None, :] // GS == np.arange(NG)[:, None]).astype(np.float32)

    def ktile(a):
        # (512, N) -> (128, 4, N): partition-major layout for one big tile
        return np.ascontiguousarray(a.reshape(CT, 128, -1).transpose(1, 0, 2))

    use_bqk = bool(np.any(bqk_host))
    use_bproj = bool(np.any(bproj_host))
    biases = np.concatenate([bqk_host, bproj_host, gamma_host, beta_host], axis=1)

    common = {
        "wqk": ktile(wqk_host),
        "wv": ktile(wvT_host),
        "wproj": ktile(wprojT_host),
        "gsel": ktile(gsel_host),
        "gselt": np.ascontiguousarray(gselt_host),
        "biases": np.ascontiguousarray(biases.astype(np.float32)),
    }
    return x, common, use_bqk, use_bproj


def kernel(x, gamma, beta, w_qkv, b_qkv, w_proj, b_proj):
    b, c, H, Wd = x.shape
    assert (b, c, H, Wd) == (8, C, 32, 32)
    xf, common, use_bqk, use_bproj = make_host_inputs(
        x, gamma, beta, w_qkv, b_qkv, w_proj, b_proj)
    xf = xf.reshape(b, C, T)

    nc = _get_program(use_bqk, use_bproj)
    if not getattr(nc, "_waits_split", False):
        _split_multi_waits(nc)
        nc._waits_split = True
    in_maps = [dict(common, x=np.ascontiguousarray(xf[i])) for i in range(NCORES)]
    res = run_bass_kernel_spmd(nc, in_maps, list(range(NCORES)))
    out = np.stack([res.results[i]["out"] for i in range(NCORES)])
    return out.reshape(b, C, H, Wd).astype(np.float32)


if __name__ == "__main__":
    rng = np.random.default_rng(0)
    args = {
        "x": rng.standard_normal((8, C, 32, 32), dtype=np.float32),
        "gamma": np.ones(C, np.float32),
        "beta": np.zeros(C, np.float32),
        "w_qkv": (rng.standard_normal((3 * C, C)) * 0.02).astype(np.float32),
        "b_qkv": np.zeros(3 * C, np.float32),
        "w_proj": (rng.standard_normal((C, C)) * 0.02).astype(np.float32),
        "b_proj": np.zeros(C, np.float32),
    }
    out = kernel(**args)
    print(out.shape, out.dtype)


# revision 29
# speedup vs baseline: 1.4377x; 1.1930x over previous
"""AttentionBlock kernel for 8 Trainium2 NeuronCores.

Problem: x(8,512,32,32) -> GroupNorm(32) -> qkv 1x1 conv -> 8-head attention
         over T=1024 tokens -> proj 1x1 conv -> residual.

Sharding: pure data parallel - one batch element per core, no collectives.

Per-core dataflow (c=512 channels on partitions, T=1024 tokens on free dim):
  1. GroupNorm stats: bn_stats per channel; tiny PE matmuls (group-selector
     matrices) aggregate across the 16 channels of each group and broadcast
     group stats back to channels; xn = x*A + B via one tensor_scalar per tile.
  2. q,k projection (fp32r) with head-pair-permuted weights; outputs cast to
     bf16: q/k of heads (2p,2p+1) stacked on partitions 0-63/64-127 of one
     (128,1024) tile each, so the K=64 score matmuls of a pair land in
     disjoint PE row groups.
  3. v is produced TRANSPOSED directly by the matmul vT = xn^T @ wvT
     (lhsT = xn), avoiding any explicit transpose for the second attention
     matmul; cast to bf16 with an all-ones column appended (M=65) so the WV
     matmul also emits the softmax denominator D[t] as output row 64.
  4. scores^T(s,t) = k^T q per head in bf16 (hw runs K<128 fp32r matmuls at
     half rate; bf16 runs at full rate and the ~4e-3 rounding is well within
     tolerance).
  5. exp on the scalar engine reading PSUM, writing bf16 (scale=0.125 folds
     the attention scaling exactly); WV accumulates hu (and D at row 64) in
     fp32 PSUM. 1/D via reciprocal_approx_fast after a K=1 ones-matmul
     broadcasts D to all partitions.
  6. proj (fp32r) + residual into the x tiles, DMA out.

Schedule: WV runs 3 pipeline steps behind scores/exp so the scalar engine
never starves; the previous pair's division chain is emitted inside the next
pair (keeps the in-order PE queue from stalling on it); qk chunks 2..7 and
all vT matmuls are spread across pair 0's pipeline steps.
"""

import sys

for _p in ("/opt/trn_rl_repo",):
    if _p not in sys.path:
        sys.path.insert(0, _p)

import numpy as np

import concourse.bass as bass
import concourse.tile as tile
from concourse import mybir
from concourse.bass_utils import run_bass_kernel_spmd
from concourse.vector_clock import ScopedClock, VectorClock


def _patched_drain_and_barrier(self, tick_clock, wait_clock):
    # This container's walrus rejects instructions carrying more than one
    # sync wait. Split the final drain's global-clock waits across NOPs.
    g = tick_clock.global_clock
    n = len(g)
    for lo in range(0, n, 4):
        vec = [g[p] if lo <= p < lo + 4 else 0 for p in range(n)]
        if not any(vec):
            continue
        nop_inst = self.nc.sync.nop()
        wait_clock.add_sem_waits(nop_inst.ins, ScopedClock({None: VectorClock(vec)}))
    self.nc.sync.drain()
    self.nc.all_engine_barrier()
    assert self.sems is not None
    popped = self.nc._tile_sem_poison_stack.pop()
    assert popped is self._sem_poison
    self.nc.clear_and_free_semaphores(list(self.sems.allocated().values()))
    self.nc.all_engine_barrier()


tile.TileContext._drain_and_barrier = _patched_drain_and_barrier


def _split_multi_waits(nc):
    # This walrus build accepts at most one sync wait per instruction. Hoist
    # surplus waits onto same-engine NOPs placed immediately before.
    n = 0
    for fn in nc.m.functions:
        for blk in fn.blocks:
            out = []
            for inst in blk.instructions:
                si = inst.sync_info
                if si is not None and si.on_wait and len(si.on_wait) > 1:
                    waits = list(si.on_wait)
                    for w in waits[:-1]:
                        nop = mybir.InstNoOp(name=f"{inst.name}_w{n}", ins=[], outs=[])
                        n += 1
                        nop.engine = inst.engine
                        nop.sync_info = mybir.SyncInfo(on_wait=[w], on_update=[])
                        out.append(nop)
                    si.on_wait = [waits[-1]]
                out.append(inst)
            blk.instructions = out


F32 = mybir.dt.float32
F32R = mybir.dt.float32r
BF16 = mybir.dt.bfloat16

C = 512
T = 1024
NH = 8          # heads
CH = C // NH    # 64 channels per head
NG = 32         # groups
GS = C // NG    # 16 channels per group
EPS = 1e-5
NCORES = 8
CT = C // 128   # 4 channel tiles
ST = T // 128   # 8 s tiles
TC = T // 512   # 2 t chunks
DELAY = 4       # wv runs this many pipeline steps behind scores/exp


def build_program(use_bqk: bool, use_bproj: bool) -> bass.Bass:
    nc = bass.Bass()

    x_d = nc.dram_tensor("x", [C, T], F32R, kind="ExternalInput")
    wqk_d = nc.dram_tensor("wqk", [128, CT, 2 * C], F32R, kind="ExternalInput")
    wv_d = nc.dram_tensor("wv", [128, CT, C], F32R, kind="ExternalInput")
    wproj_d = nc.dram_tensor("wproj", [128, CT, C], F32R, kind="ExternalInput")
    gsel_d = nc.dram_tensor("gsel", [128, CT, NG], F32R, kind="ExternalInput")
    gselt_d = nc.dram_tensor("gselt", [NG, C], F32R, kind="ExternalInput")
    # combined small params: [bqk(8) | bproj(4) | gamma(4) | beta(4)]
    bias_d = nc.dram_tensor("biases", [128, 20], F32, kind="ExternalInput")
    out_d = nc.dram_tensor("out", [C, T], F32R, kind="ExternalOutput")

    with tile.TileContext(nc) as tc:
        with (
            tc.tile_pool(name="persist", bufs=1) as P,
            tc.tile_pool(name="work", bufs=2) as W,
            tc.tile_pool(name="ps", bufs=2, space="PSUM") as PS,
            tc.tile_pool(name="dr", bufs=2, space="DRAM") as DR,
        ):
            _body(nc, tc, P, W, PS, DR, locals(), use_bqk, use_bproj)
    return nc


def _body(nc, tc, P, W, PS, DR, d, use_bqk, use_bproj):
    x_d, wqk_d, wv_d, wproj_d = d["x_d"], d["wqk_d"], d["wv_d"], d["wproj_d"]
    gsel_d, gselt_d, bias_d, out_d = d["gsel_d"], d["gselt_d"], d["bias_d"], d["out_d"]

    # ---- persistent SBUF tiles ----
    x_t = [P.tile([128, T], F32R, name=f"x{i}", tag=f"x{i}") for i in range(CT)]
    xn_t = [P.tile([128, T], F32R, name=f"xn{i}", tag=f"xn{i}") for i in range(CT)]
    h_t = [P.tile([128, T], F32R, name=f"h{i}", tag=f"h{i}") for i in range(CT)]
    q_t = [P.tile([128, T], BF16, name=f"q{p}", tag=f"q{p}") for p in range(4)]
    k_t = [P.tile([128, T], BF16, name=f"k{p}", tag=f"k{p}") for p in range(4)]
    vaug_t = [P.tile([128, NH, CH + 1], BF16, name=f"va{s}", tag=f"va{s}") for s in range(ST)]
    wqk_t = P.tile([128, CT, 2 * C], F32R, name="wqk_t")
    wv_t = P.tile([128, CT, C], F32R, name="wv_t")
    wp_t = P.tile([128, CT, C], F32R, name="wp_t")
    gsel_t = P.tile([128, CT, NG], F32R, name="gsel_t")
    gselt_t = P.tile([NG, C], F32R, name="gselt_t")
    ab_t = [P.tile([128, 2], F32, name=f"ab{i}", tag=f"ab{i}") for i in range(CT)]
    bias_sb = P.tile([128, 20], F32, name="bias_sb")
    bqk_sb = bias_sb[:, 0:8]
    bproj_sb = bias_sb[:, 8:12]
    gamma_sb = bias_sb[:, 12:16]
    beta_sb = bias_sb[:, 16:20]
    ones_t = P.tile([128, 128], F32R, name="ones_t")
    eps_sb = P.tile([NG, 1], F32, name="eps_sb")
    mvr_sb = P.tile([NG, 2], F32R, name="mvr_sb")

    # ---- input DMAs: x on sync, weights on the gpsimd SWDGE queues ----
    # (separate semaphore pool, so a big weight transfer never blocks an
    # x tile's HWDGE slot)
    x_dmas = []
    for i in range(CT):
        for half in range(2):
            fs = slice(half * 512, half * 512 + 512)
            x_dmas.append(nc.sync.dma_start(
                out=x_t[i][:, fs], in_=x_d[i * 128:(i + 1) * 128, fs]))
    nc.gpsimd.dma_start(out=gsel_t, in_=gsel_d[:, :, :])
    nc.gpsimd.dma_start(out=gselt_t, in_=gselt_d[:, :])
    nc.gpsimd.dma_start(out=bias_sb, in_=bias_d[:, :])
    # weight transfers start only after x has landed: stats (and the whole
    # pipeline behind them) need x first, and HBM bandwidth is the startup
    # bottleneck. wqk is split so pair 0's q/k columns land first.
    w_dmas = [
        nc.gpsimd.dma_start(out=wqk_t[:, :, 0:256], in_=wqk_d[:, :, 0:256]),
        nc.gpsimd.dma_start(out=wqk_t[:, :, 256:2 * C], in_=wqk_d[:, :, 256:2 * C]),
        nc.gpsimd.dma_start(out=wv_t, in_=wv_d[:, :, :]),
        nc.gpsimd.dma_start(out=wp_t, in_=wproj_d[:, :, :]),
    ]
    for w in w_dmas:
        bass._add_dep_helper(w.ins, x_dmas[-1].ins, sync=True,
                             reason="weights yield HBM bandwidth to x")
    nc.gpsimd.memset(ones_t.bitcast(F32), 1.0)
    nc.vector.memset(eps_sb, EPS)
    # dummy op pulls the natural_log+exp ACT table set in while everything
    # else is still loading; every transcendental in this kernel (exp, ln,
    # and rsqrt spelled as exp(-ln/2)) lives in that one set
    warm = W.tile([NG, 1], F32, name="warm", tag="warm", bufs=1)
    nc.scalar.activation(out=warm, in_=eps_sb,
                         func=mybir.ActivationFunctionType.Exp)

    # ---- phase 1: group norm statistics ----
    # per-channel sum via DVE tensor_scalar accumulate (dumps into xn, which
    # is fully overwritten later) and sum(x^2) via ACT Square accumulate
    # (dumps into h_t, dead until attention); both run while x halves land
    mv32_ps = PS.tile([NG, 2], F32, name="mv32_ps", tag="sc")
    for i in range(CT):
        st2 = W.tile([128, 2], F32R, name="st2", tag="st2", bufs=4)
        with nc.allow_low_precision(reason="f32r accum tags; same 4B payload"):
            nc.vector.tensor_scalar(out=xn_t[i], in0=x_t[i],
                                    scalar1=1.0, scalar2=0.0,
                                    op0=mybir.AluOpType.mult,
                                    op1=mybir.AluOpType.add,
                                    accum_out=st2[:, 0:1])
            nc.scalar.activation(out=h_t[i], in_=x_t[i],
                                 func=mybir.ActivationFunctionType.Square,
                                 accum_out=st2[:, 1:2])
        # group aggregation: (1/(16*1024)) * indicator^T @ [sum, sumsq]
        nc.tensor.matmul(out=mv32_ps, lhsT=gsel_t[:, i, :], rhs=st2,
                         start=(i == 0), stop=(i == CT - 1))
    mv32_sb = W.tile([NG, 2], F32, name="mv32_sb", tag="mv32", bufs=1)
    sdtmp = W.tile([NG, 2], F32, name="sdtmp", tag="sdtmp", bufs=1)
    nc.vector.tensor_copy(out=mv32_sb, in_=mv32_ps)
    # var_g = E[x^2]_g - mean_g^2 ; rstd = 1/sqrt(var+eps)
    nc.vector.tensor_mul(out=sdtmp[:, 0:1], in0=mv32_sb[:, 0:1], in1=mv32_sb[:, 0:1])
    nc.vector.tensor_sub(out=sdtmp[:, 1:2], in0=mv32_sb[:, 1:2], in1=sdtmp[:, 0:1])
    nc.scalar.activation(out=sdtmp[:, 1:2], in_=sdtmp[:, 1:2],
                         func=mybir.ActivationFunctionType.Ln,
                         bias=eps_sb, scale=1.0)
    with nc.allow_low_precision(reason="f32r tag on rstd; same 4-byte payload"):
        nc.scalar.activation(out=mvr_sb[:, 1:2], in_=sdtmp[:, 1:2],
                             func=mybir.ActivationFunctionType.Exp,
                             scale=-0.5)
    nc.vector.tensor_copy(out=mvr_sb[:, 0:1], in_=mv32_sb[:, 0:1])
    # broadcast group stats back to channels, per-channel A/B, xn = x*A + B
    for i in range(CT):
        mr_ps = PS.tile([128, 2], F32, name="mr_ps", tag="sc")
        nc.tensor.matmul(out=mr_ps, lhsT=gselt_t[:, i * 128:(i + 1) * 128],
                         rhs=mvr_sb, start=True, stop=True)
        abm = W.tile([128, 1], F32, name="abm", tag="abm", bufs=4)
        nc.vector.tensor_mul(out=ab_t[i][:, 0:1], in0=mr_ps[:, 1:2], in1=gamma_sb[:, i:i + 1])
        nc.vector.tensor_mul(out=abm, in0=mr_ps[:, 0:1], in1=ab_t[i][:, 0:1])
        nc.vector.tensor_sub(out=ab_t[i][:, 1:2], in0=beta_sb[:, i:i + 1], in1=abm)
        nc.vector.tensor_scalar(out=xn_t[i], in0=x_t[i],
                                scalar1=ab_t[i][:, 0:1], scalar2=ab_t[i][:, 1:2],
                                op0=mybir.AluOpType.mult, op1=mybir.AluOpType.add)

    # ---- deferred PE work: 4-matmul units spread across pairs 0..2 ----
    def qk_unit(oc, t2):
        p, is_k = oc // 2, oc % 2
        fs = slice(t2 * 512, t2 * 512 + 512)
        ps = PS.tile([128, 512], F32, name="qkps", tag="sc")
        for kk in range(CT):
            nc.tensor.matmul(out=ps,
                             lhsT=wqk_t[:, kk, oc * 128:(oc + 1) * 128],
                             rhs=xn_t[kk][:, fs],
                             start=(kk == 0), stop=(kk == CT - 1))
        dst = (k_t[p] if is_k else q_t[p])[:, fs]
        if use_bqk:
            nc.vector.tensor_scalar(out=dst, in0=ps, scalar1=bqk_sb[:, oc:oc + 1],
                                    scalar2=None, op0=mybir.AluOpType.add)
        else:
            nc.vector.tensor_copy(out=dst, in_=ps)

    def vt_unit(s):
        # whole-tile memset (strided ones-column memset fails this walrus);
        # the copy below overwrites cols 0..63 per head, col 64 stays 1.0
        nc.vector.memset(vaug_t[s], 1.0)
        ps = PS.tile([128, C], F32, name="vtps", tag="sc")
        for kk in range(CT):
            nc.tensor.matmul(out=ps,
                             lhsT=xn_t[kk][:, s * 128:(s + 1) * 128],
                             rhs=wv_t[:, kk, :],
                             start=(kk == 0), stop=(kk == CT - 1))
        nc.vector.tensor_copy(out=vaug_t[s][:, :, 0:CH],
                              in_=ps.rearrange("p (h e) -> p h e", e=CH))

    for oc in range(2):
        for t2 in range(TC):
            qk_unit(oc, t2)
    # (pair, ss) -> work units; vT for s must land by the position where
    # wv consumes vaug[s] (DELAY steps after scores s, counting the spill
    # into the next pair); qk for pair p anywhere before pair p
    sched = {}
    for s in range(6):
        sched.setdefault((0, s), []).append(lambda s=s: vt_unit(s))
    sched.setdefault((1, 0), []).append(lambda: vt_unit(6))
    sched.setdefault((1, 1), []).append(lambda: vt_unit(7))
    for j, (oc, t2) in enumerate([(oc, t2) for oc in (2, 3) for t2 in range(TC)]):
        sched.setdefault((0, 6 + j // 2), []).append(lambda oc=oc, t2=t2: qk_unit(oc, t2))
    for j, (oc, t2) in enumerate([(oc, t2) for oc in (4, 5) for t2 in range(TC)]):
        sched.setdefault((1, 2 + j), []).append(lambda oc=oc, t2=t2: qk_unit(oc, t2))
    for j, (oc, t2) in enumerate([(oc, t2) for oc in (6, 7) for t2 in range(TC)]):
        sched.setdefault((2, j), []).append(lambda oc=oc, t2=t2: qk_unit(oc, t2))

    # ---- attention: head pairs, wv DELAY steps behind scores/exp ----
    def division(p, hu_ps, htmp):
        """h = hu/D for pair p, into h_t[p].

        1/D on DVE costs 8 cycles/element of FREE dim, so the four D rows
        (4 x 512) are DMA-reshaped to (64,32) first: one 0.4us reciprocal.
        The reciprocal then takes a DRAM round trip so a partition-broadcast
        DMA (stride-0 partition reads are DRAM-only) can replicate each
        512-vector across 64 partitions for the final psum*sbuf multiply.
        Nothing lands on the scalar engine (pacing exp) or PE.
        """
        chains = [(half, t2) for half in range(2) for t2 in range(TC)]
        scr = DR.tile([4, 512], F32, name="scr", tag="scr", bufs=2)
        scr2 = DR.tile([4, 512], F32, name="scr2", tag="scr2", bufs=2)
        dall = W.tile([64, 32], F32, name="dall", tag="dall", bufs=2)
        ralt = W.tile([64, 32], F32, name="ralt", tag="ralt", bufs=2)
        husb = []
        for j, (half, t2) in enumerate(chains):
            hs = W.tile([128, 512], F32, name="husb", tag="husb", bufs=8)
            husb.append(hs)
            # copy h rows + D row out of PSUM so the accumulate banks free
            # immediately instead of waiting out the division's DMA latency
            nc.vector.tensor_copy(out=hs[0:CH + 1, :],
                                  in_=hu_ps[half][t2][0:CH + 1, :])
            nc.sync.dma_start(out=scr[j:j + 1, :], in_=hs[CH:CH + 1, :])
        nc.sync.dma_start(out=dall,
                          in_=scr.rearrange("a (b c) -> (a b) c", c=32))
        nc.vector.reciprocal(out=ralt, in_=dall)
        nc.sync.dma_start(out=scr2.rearrange("a (b c) -> (a b) c", c=32),
                          in_=ralt)
        for j, (half, t2) in enumerate(chains):
            fs = slice(t2 * 512, t2 * 512 + 512)
            rcb = W.tile([64, 512], F32, name="rcb", tag="rcb", bufs=4)
            row = scr2[j:j + 1, :]
            bcast = bass.AP(tensor=row.tensor, offset=row.offset,
                            ap=[[0, 64], list(row.ap[-1])])
            nc.sync.dma_start(out=rcb, in_=bcast)
            dst = h_t[p] if half == 0 else htmp
            with nc.allow_low_precision(reason="f32r tag on h; same payload"):
                nc.vector.tensor_mul(out=dst[0:CH, fs],
                                     in0=husb[j][0:CH, :],
                                     in1=rcb)
        # odd head's h goes to partitions 64..127 (cross-partition -> DMA)
        nc.sync.dma_start(out=h_t[p][64:128, :], in_=htmp[0:64, :])

    def division_act(p, hu_ps, htmp):
        """Tail variant: 1/D = exp(-ln D) on the (by now idle) scalar engine
        after a K=1 ones-matmul broadcasts D across partitions - avoids the
        ~10us of DMA round-trip latency the DMA-based division would expose
        at the end of the kernel."""
        for half in range(2):
            for t2 in range(TC):
                fs = slice(t2 * 512, t2 * 512 + 512)
                dsb = W.tile([128, 512], F32R, name="dsbt", tag="dsb", bufs=4)
                nc.vector.tensor_copy(out=dsb[CH:CH + 1, :],
                                      in_=hu_ps[half][t2][CH:CH + 1, :])
                bc = PS.tile([128, 512], F32, name="bcps", tag="sc")
                nc.tensor.matmul(out=bc, lhsT=ones_t[CH:CH + 1, :],
                                 rhs=dsb[CH:CH + 1, :], start=True, stop=True)
                lnd = W.tile([128, 512], F32, name="lnd", tag="lnd", bufs=2)
                nc.scalar.activation(out=lnd, in_=bc,
                                     func=mybir.ActivationFunctionType.Ln)
                rc = W.tile([128, 512], F32, name="rct", tag="rct", bufs=2)
                nc.scalar.activation(out=rc, in_=lnd,
                                     func=mybir.ActivationFunctionType.Exp,
                                     scale=-1.0)
                dst = h_t[p] if half == 0 else htmp
                with nc.allow_low_precision(reason="f32r tag on h; same payload"):
                    nc.vector.tensor_mul(out=dst[0:CH, fs],
                                         in0=hu_ps[half][t2][0:CH, :],
                                         in1=rc[0:CH, :])
        nc.sync.dma_start(out=h_t[p][64:128, :], in_=htmp[0:64, :])

    pend = None  # (p, hu_ps, htmp) awaiting division
    for p in range(4):
        hu_ps = [[None] * TC for _ in range(2)]
        htmp = W.tile([64, T], F32R, name="htmp", tag="htmp", bufs=2)
        expw = {}
        prev = pend
        for ss in range(ST):
            scps = [None, None]
            for half in range(2):
                base = 64 * half
                sc = PS.tile([128, T], F32, name=f"scps{half}", tag="sc")
                scps[half] = sc
                for t2 in range(TC):
                    fs = slice(t2 * 512, t2 * 512 + 512)
                    nc.tensor.matmul(
                        out=sc[:, fs],
                        lhsT=k_t[p][base:base + 64, ss * 128:(ss + 1) * 128],
                        rhs=q_t[p][base:base + 64, fs],
                        start=True, stop=True)
                ew = W.tile([128, T], BF16, name="expw", tag="expw", bufs=14)
                nc.scalar.activation(out=ew, in_=scps[half],
                                     func=mybir.ActivationFunctionType.Exp,
                                     scale=0.125)
                expw[half, ss] = ew
            for unit in sched.get((p, ss), []):
                unit()
            if prev is not None:
                # previous pair's wv tail (ss ST-DELAY..ST-1), then division
                if ss < DELAY:
                    _wv(nc, PS, vaug_t, prev[3], prev[1], prev[0], ST - DELAY + ss)
                if ss == DELAY - 1:
                    division(prev[0], prev[1], prev[2])
                    prev = pend = None
            if ss >= DELAY:
                _wv(nc, PS, vaug_t, expw, hu_ps, p, ss - DELAY)
        pend = (p, hu_ps, htmp, expw)
    for ss in range(ST - DELAY, ST):
        _wv(nc, PS, vaug_t, pend[3], pend[1], pend[0], ss)
    division_act(pend[0], pend[1], pend[2])

    # ---- proj + residual ----
    for oc in range(CT):
        ps = PS.tile([128, T], F32, name="prps", tag="sc")
        for t2 in range(TC):
            fs = slice(t2 * 512, t2 * 512 + 512)
            for kk in range(CT):
                nc.tensor.matmul(out=ps[:, fs],
                                 lhsT=wp_t[:, kk, oc * 128:(oc + 1) * 128],
                                 rhs=h_t[kk][:, fs],
                                 start=(kk == 0), stop=(kk == CT - 1))
        if use_bproj:
            prtmp = W.tile([128, T], F32, name="prtmp", tag="prtmp", bufs=2)
            nc.vector.tensor_scalar(out=prtmp, in0=ps, scalar1=bproj_sb[:, oc:oc + 1],
                                    scalar2=None, op0=mybir.AluOpType.add)
            with nc.allow_low_precision(reason="f32r tag on out; same payload"):
                nc.vector.tensor_add(out=x_t[oc], in0=x_t[oc], in1=prtmp)
        else:
            with nc.allow_low_precision(reason="f32r tag on out; same payload"):
                nc.vector.tensor_add(out=x_t[oc], in0=x_t[oc], in1=ps)
        nc.sync.dma_start(out=out_d[oc * 128:(oc + 1) * 128, :], in_=x_t[oc])


def _wv(nc, PS, vaug_t, expw, hu_ps, p, ss):
    for half in range(2):
        h = 2 * p + half
        for t2 in range(TC):
            if ss == 0:
                hu_ps[half][t2] = PS.tile([128, 512], F32,
                                          name="hups", tag="hu", bufs=4)
            fs = slice(t2 * 512, t2 * 512 + 512)
            nc.tensor.matmul(out=hu_ps[half][t2][0:CH + 1, :],
                             lhsT=vaug_t[ss][:, h, :],
                             rhs=expw[half, ss][:, fs],
                             start=(ss == 0), stop=(ss == ST - 1))


_PROGRAM_CACHE = {}


def _get_program(use_bqk, use_bproj):
    key = (use_bqk, use_bproj)
    if key not in _PROGRAM_CACHE:
        _PROGRAM_CACHE[key] = build_program(*key)
    return _PROGRAM_CACHE[key]


def make_host_inputs(x, gamma, beta, w_qkv, b_qkv, w_proj, b_proj):
    """Host-side preprocessing shared by all cores."""
    x = np.asarray(x, np.float32)
    w_qkv = np.asarray(w_qkv, np.float32)
    b_qkv = np.asarray(b_qkv, np.float32)
    w_proj = np.asarray(w_proj, np.float32)
    b_proj = np.asarray(b_proj, np.float32)
    gamma = np.asarray(gamma, np.float32)
    beta = np.asarray(beta, np.float32)

    # per-head slices of w_qkv rows (3c, c): head h -> q,k,v at 192h+{0,64,128}
    wq = np.stack([w_qkv[192 * h:192 * h + 64] for h in range(NH)])
    wk = np.stack([w_qkv[192 * h + 64:192 * h + 128] for h in range(NH)])
    wv = np.stack([w_qkv[192 * h + 128:192 * h + 192] for h in range(NH)])
    bq = np.stack([b_qkv[192 * h:192 * h + 64] for h in range(NH)])
    bk = np.stack([b_qkv[192 * h + 64:192 * h + 128] for h in range(NH)])
    bv = np.stack([b_qkv[192 * h + 128:192 * h + 192] for h in range(NH)])

    # wqk (512c, 1024): chunk 2p = q of heads (2p,2p+1), chunk 2p+1 = k of same
    chunks, bqk_chunks = [], []
    for p in range(4):
        chunks.append(np.concatenate([wq[2 * p], wq[2 * p + 1]], 0).T)
        chunks.append(np.concatenate([wk[2 * p], wk[2 * p + 1]], 0).T)
        bqk_chunks.append(np.concatenate([bq[2 * p], bq[2 * p + 1]], 0))
        bqk_chunks.append(np.concatenate([bk[2 * p], bk[2 * p + 1]], 0))
    wqk_host = np.concatenate(chunks, axis=1)                     # (512,1024)
    bqk_host = np.stack(bqk_chunks, axis=1)                       # (128,8)

    wvT_host = wv.reshape(C, C).T.copy()                          # (512c, 512vch)
    wprojT_host = w_proj.T.copy()                                 # (512c, 512o)
    # v-bias contributes exactly b_v through the softmax (rows sum to 1);
    # fold it into the proj bias
    bproj_eff = b_proj + w_proj @ bv.reshape(C)
    bproj_host = bproj_eff.reshape(CT, 128).T.copy()
    gamma_host = gamma.reshape(CT, 128).T.copy()
    beta_host = beta.reshape(CT, 128).T.copy()

    cidx = np.arange(C)
    # gsel aggregates raw [sum, sum(x^2)] rows -> per-group means
    gsel_host = (cidx[:, None] // GS == np.arange(NG)[None, :]).astype(np.float32) / (GS * T)
    gselt_host = (cidx[None, :] // GS == np.arange(NG)[:, None]).astype(np.float32)

    def ktile(a):
        # (512, N) -> (128, 4, N): partition-major layout for one big tile
        return np.ascontiguousarray(a.reshape(CT, 128, -1).transpose(1, 0, 2))

    use_bqk = bool(np.any(bqk_host))
    use_bproj = bool(np.any(bproj_host))
    biases = np.concatenate([bqk_host, bproj_host, gamma_host, beta_host], axis=1)

    common = {
        "wqk": ktile(wqk_host),
        "wv": ktile(wvT_host),
        "wproj": ktile(wprojT_host),
        "gsel": ktile(gsel_host),
        "gselt": np.ascontiguousarray(gselt_host),
        "biases": np.ascontiguousarray(biases.astype(np.float32)),
    }
    return x, common, use_bqk, use_bproj


def kernel(x, gamma, beta, w_qkv, b_qkv, w_proj, b_proj):
    b, c, H, Wd = x.shape
    assert (b, c, H, Wd) == (8, C, 32, 32)
    xf, common, use_bqk, use_bproj = make_host_inputs(
        x, gamma, beta, w_qkv, b_qkv, w_proj, b_proj)
    xf = xf.reshape(b, C, T)

    nc = _get_program(use_bqk, use_bproj)
    if not getattr(nc, "_waits_split", False):
        _split_multi_waits(nc)
        nc._waits_split = True
    in_maps = [dict(common, x=np.ascontiguousarray(xf[i])) for i in range(NCORES)]
    res = run_bass_kernel_spmd(nc, in_maps, list(range(NCORES)))
    out = np.stack([res.results[i]["out"] for i in range(NCORES)])
    return out.reshape(b, C, H, Wd).astype(np.float32)


if __name__ == "__main__":
    rng = np.random.default_rng(0)
    args = {
        "x": rng.standard_normal((8, C, 32, 32), dtype=np.float32),
        "gamma": np.ones(C, np.float32),
        "beta": np.zeros(C, np.float32),
        "w_qkv": (rng.standard_normal((3 * C, C)) * 0.02).astype(np.float32),
        "b_qkv": np.zeros(3 * C, np.float32),
        "w_proj": (rng.standard_normal((C, C)) * 0.02).astype(np.float32),
        "b_proj": np.zeros(C, np.float32),
    }
    out = kernel(**args)
    print(out.shape, out.dtype)


# revision 30
# speedup vs baseline: 1.4625x; 1.0172x over previous
"""AttentionBlock kernel for 8 Trainium2 NeuronCores.

Problem: x(8,512,32,32) -> GroupNorm(32) -> qkv 1x1 conv -> 8-head attention
         over T=1024 tokens -> proj 1x1 conv -> residual.

Sharding: pure data parallel - one batch element per core, no collectives.

Per-core dataflow (c=512 channels on partitions, T=1024 tokens on free dim):
  1. GroupNorm stats: bn_stats per channel; tiny PE matmuls (group-selector
     matrices) aggregate across the 16 channels of each group and broadcast
     group stats back to channels; xn = x*A + B via one tensor_scalar per tile.
  2. q,k projection (fp32r) with head-pair-permuted weights; outputs cast to
     bf16: q/k of heads (2p,2p+1) stacked on partitions 0-63/64-127 of one
     (128,1024) tile each, so the K=64 score matmuls of a pair land in
     disjoint PE row groups.
  3. v is produced TRANSPOSED directly by the matmul vT = xn^T @ wvT
     (lhsT = xn), avoiding any explicit transpose for the second attention
     matmul; cast to bf16 with an all-ones column appended (M=65) so the WV
     matmul also emits the softmax denominator D[t] as output row 64.
  4. scores^T(s,t) = k^T q per head in bf16 (hw runs K<128 fp32r matmuls at
     half rate; bf16 runs at full rate and the ~4e-3 rounding is well within
     tolerance).
  5. exp on the scalar engine reading PSUM, writing bf16 (scale=0.125 folds
     the attention scaling exactly); WV accumulates hu (and D at row 64) in
     fp32 PSUM. 1/D via reciprocal_approx_fast after a K=1 ones-matmul
     broadcasts D to all partitions.
  6. proj (fp32r) + residual into the x tiles, DMA out.

Schedule: WV runs 3 pipeline steps behind scores/exp so the scalar engine
never starves; the previous pair's division chain is emitted inside the next
pair (keeps the in-order PE queue from stalling on it); qk chunks 2..7 and
all vT matmuls are spread across pair 0's pipeline steps.
"""

import sys

for _p in ("/opt/trn_rl_repo",):
    if _p not in sys.path:
        sys.path.insert(0, _p)

import numpy as np

import concourse.bass as bass
import concourse.tile as tile
from concourse import mybir
from concourse.bass_utils import run_bass_kernel_spmd
from concourse.vector_clock import ScopedClock, VectorClock


def _patched_drain_and_barrier(self, tick_clock, wait_clock):
    # This container's walrus rejects instructions carrying more than one
    # sync wait. Split the final drain's global-clock waits across NOPs.
    g = tick_clock.global_clock
    n = len(g)
    for lo in range(0, n, 4):
        vec = [g[p] if lo <= p < lo + 4 else 0 for p in range(n)]
        if not any(vec):
            continue
        nop_inst = self.nc.sync.nop()
        wait_clock.add_sem_waits(nop_inst.ins, ScopedClock({None: VectorClock(vec)}))
    self.nc.sync.drain()
    self.nc.all_engine_barrier()
    assert self.sems is not None
    popped = self.nc._tile_sem_poison_stack.pop()
    assert popped is self._sem_poison
    self.nc.clear_and_free_semaphores(list(self.sems.allocated().values()))
    self.nc.all_engine_barrier()


tile.TileContext._drain_and_barrier = _patched_drain_and_barrier


def _split_multi_waits(nc):
    # This walrus build accepts at most one sync wait per instruction. Hoist
    # surplus waits onto same-engine NOPs placed immediately before.
    n = 0
    for fn in nc.m.functions:
        for blk in fn.blocks:
            out = []
            for inst in blk.instructions:
                si = inst.sync_info
                if si is not None and si.on_wait and len(si.on_wait) > 1:
                    waits = list(si.on_wait)
                    for w in waits[:-1]:
                        nop = mybir.InstNoOp(name=f"{inst.name}_w{n}", ins=[], outs=[])
                        n += 1
                        nop.engine = inst.engine
                        nop.sync_info = mybir.SyncInfo(on_wait=[w], on_update=[])
                        out.append(nop)
                    si.on_wait = [waits[-1]]
                out.append(inst)
            blk.instructions = out


F32 = mybir.dt.float32
F32R = mybir.dt.float32r
BF16 = mybir.dt.bfloat16

C = 512
T = 1024
NH = 8          # heads
CH = C // NH    # 64 channels per head
NG = 32         # groups
GS = C // NG    # 16 channels per group
EPS = 1e-5
NCORES = 8
CT = C // 128   # 4 channel tiles
ST = T // 128   # 8 s tiles
TC = T // 512   # 2 t chunks
DELAY = 4       # wv runs this many pipeline steps behind scores/exp


def build_program(use_bqk: bool, use_bproj: bool) -> bass.Bass:
    nc = bass.Bass()

    x_d = nc.dram_tensor("x", [C, T], F32R, kind="ExternalInput")
    wqk_d = nc.dram_tensor("wqk", [128, CT, 2 * C], BF16, kind="ExternalInput")
    wv_d = nc.dram_tensor("wv", [128, CT, C], BF16, kind="ExternalInput")
    wproj_d = nc.dram_tensor("wproj", [128, CT, C], BF16, kind="ExternalInput")
    gsel_d = nc.dram_tensor("gsel", [128, CT, NG], F32R, kind="ExternalInput")
    gselt_d = nc.dram_tensor("gselt", [NG, C], F32R, kind="ExternalInput")
    # combined small params: [bqk(8) | bproj(4) | gamma(4) | beta(4)]
    bias_d = nc.dram_tensor("biases", [128, 20], F32, kind="ExternalInput")
    out_d = nc.dram_tensor("out", [C, T], F32R, kind="ExternalOutput")

    with tile.TileContext(nc) as tc:
        with (
            tc.tile_pool(name="persist", bufs=1) as P,
            tc.tile_pool(name="work", bufs=2) as W,
            tc.tile_pool(name="ps", bufs=2, space="PSUM") as PS,
            tc.tile_pool(name="dr", bufs=2, space="DRAM") as DR,
        ):
            _body(nc, tc, P, W, PS, DR, locals(), use_bqk, use_bproj)
    return nc


def _body(nc, tc, P, W, PS, DR, d, use_bqk, use_bproj):
    x_d, wqk_d, wv_d, wproj_d = d["x_d"], d["wqk_d"], d["wv_d"], d["wproj_d"]
    gsel_d, gselt_d, bias_d, out_d = d["gsel_d"], d["gselt_d"], d["bias_d"], d["out_d"]

    # ---- persistent SBUF tiles ----
    x_t = [P.tile([128, T], F32R, name=f"x{i}", tag=f"x{i}") for i in range(CT)]
    xn_t = [P.tile([128, T], BF16, name=f"xn{i}", tag=f"xn{i}") for i in range(CT)]
    h_t = [P.tile([128, T], BF16, name=f"h{i}", tag=f"h{i}") for i in range(CT)]
    q_t = [P.tile([128, T], BF16, name=f"q{p}", tag=f"q{p}") for p in range(4)]
    k_t = [P.tile([128, T], BF16, name=f"k{p}", tag=f"k{p}") for p in range(4)]
    vaug_t = [P.tile([128, NH, CH + 1], BF16, name=f"va{s}", tag=f"va{s}") for s in range(ST)]
    wqk_t = P.tile([128, CT, 2 * C], BF16, name="wqk_t")
    wv_t = P.tile([128, CT, C], BF16, name="wv_t")
    wp_t = P.tile([128, CT, C], BF16, name="wp_t")
    gsel_t = P.tile([128, CT, NG], F32R, name="gsel_t")
    gselt_t = P.tile([NG, C], F32R, name="gselt_t")
    ab_t = [P.tile([128, 2], F32, name=f"ab{i}", tag=f"ab{i}") for i in range(CT)]
    bias_sb = P.tile([128, 20], F32, name="bias_sb")
    bqk_sb = bias_sb[:, 0:8]
    bproj_sb = bias_sb[:, 8:12]
    gamma_sb = bias_sb[:, 12:16]
    beta_sb = bias_sb[:, 16:20]
    ones_t = P.tile([128, 128], F32R, name="ones_t")
    eps_sb = P.tile([NG, 1], F32, name="eps_sb")
    mvr_sb = P.tile([NG, 2], F32R, name="mvr_sb")

    # ---- input DMAs: x on sync, weights on the gpsimd SWDGE queues ----
    # (separate semaphore pool, so a big weight transfer never blocks an
    # x tile's HWDGE slot)
    x_dmas = []
    for i in range(CT):
        for half in range(2):
            fs = slice(half * 512, half * 512 + 512)
            x_dmas.append(nc.sync.dma_start(
                out=x_t[i][:, fs], in_=x_d[i * 128:(i + 1) * 128, fs]))
    nc.gpsimd.dma_start(out=gsel_t, in_=gsel_d[:, :, :])
    nc.gpsimd.dma_start(out=gselt_t, in_=gselt_d[:, :])
    nc.gpsimd.dma_start(out=bias_sb, in_=bias_d[:, :])
    # weight transfers start only after x has landed: stats (and the whole
    # pipeline behind them) need x first, and HBM bandwidth is the startup
    # bottleneck. wqk is split so pair 0's q/k columns land first.
    w_dmas = [
        nc.gpsimd.dma_start(out=wqk_t[:, :, 0:256], in_=wqk_d[:, :, 0:256]),
        nc.gpsimd.dma_start(out=wqk_t[:, :, 256:2 * C], in_=wqk_d[:, :, 256:2 * C]),
        nc.gpsimd.dma_start(out=wv_t, in_=wv_d[:, :, :]),
        nc.gpsimd.dma_start(out=wp_t, in_=wproj_d[:, :, :]),
    ]
    for w in w_dmas:
        bass._add_dep_helper(w.ins, x_dmas[-1].ins, sync=True,
                             reason="weights yield HBM bandwidth to x")
    nc.gpsimd.memset(ones_t.bitcast(F32), 1.0)
    nc.vector.memset(eps_sb, EPS)
    # dummy op pulls the natural_log+exp ACT table set in while everything
    # else is still loading; every transcendental in this kernel (exp, ln,
    # and rsqrt spelled as exp(-ln/2)) lives in that one set
    warm = W.tile([NG, 1], F32, name="warm", tag="warm", bufs=1)
    nc.scalar.activation(out=warm, in_=eps_sb,
                         func=mybir.ActivationFunctionType.Exp)

    # ---- phase 1: group norm statistics ----
    # per-channel sum via DVE tensor_scalar accumulate (dumps into xn, which
    # is fully overwritten later) and sum(x^2) via ACT Square accumulate
    # (dumps into h_t, dead until attention); both run while x halves land
    mv32_ps = PS.tile([NG, 4], F32, name="mv32_ps", tag="sc")
    for i in range(CT):
        st2 = W.tile([128, 4], F32R, name="st2", tag="st2", bufs=4)
        for hf in range(2):
            hs = slice(hf * 512, hf * 512 + 512)
            with nc.allow_low_precision(reason="f32r/bf16 tags; accum is f32"):
                nc.vector.tensor_scalar(out=xn_t[i][:, hs], in0=x_t[i][:, hs],
                                        scalar1=1.0, scalar2=0.0,
                                        op0=mybir.AluOpType.mult,
                                        op1=mybir.AluOpType.add,
                                        accum_out=st2[:, hf:hf + 1])
                nc.scalar.activation(out=h_t[i][:, hs], in_=x_t[i][:, hs],
                                     func=mybir.ActivationFunctionType.Square,
                                     accum_out=st2[:, 2 + hf:3 + hf])
        # group aggregation: (1/(16*1024)) * indicator^T @ [sA, sB, qA, qB]
        nc.tensor.matmul(out=mv32_ps, lhsT=gsel_t[:, i, :], rhs=st2,
                         start=(i == 0), stop=(i == CT - 1))
    mv32_sb = W.tile([NG, 4], F32, name="mv32_sb", tag="mv32", bufs=1)
    sdtmp = W.tile([NG, 2], F32, name="sdtmp", tag="sdtmp", bufs=1)
    nc.vector.tensor_copy(out=mv32_sb, in_=mv32_ps)
    # mean = sA+sB ; E[x^2] = qA+qB ; var = E[x^2] - mean^2
    with nc.allow_low_precision(reason="f32r tag on mean; same payload"):
        nc.vector.tensor_add(out=mvr_sb[:, 0:1], in0=mv32_sb[:, 0:1], in1=mv32_sb[:, 1:2])
    nc.vector.tensor_add(out=sdtmp[:, 1:2], in0=mv32_sb[:, 2:3], in1=mv32_sb[:, 3:4])
    nc.vector.tensor_mul(out=sdtmp[:, 0:1], in0=mvr_sb[:, 0:1].bitcast(F32), in1=mvr_sb[:, 0:1].bitcast(F32))
    nc.vector.tensor_sub(out=sdtmp[:, 1:2], in0=sdtmp[:, 1:2], in1=sdtmp[:, 0:1])
    # rstd = exp(-0.5*ln(var+eps)): stays inside the one ACT table set
    nc.scalar.activation(out=sdtmp[:, 1:2], in_=sdtmp[:, 1:2],
                         func=mybir.ActivationFunctionType.Ln,
                         bias=eps_sb, scale=1.0)
    with nc.allow_low_precision(reason="f32r tag on rstd; same 4-byte payload"):
        nc.scalar.activation(out=mvr_sb[:, 1:2], in_=sdtmp[:, 1:2],
                             func=mybir.ActivationFunctionType.Exp,
                             scale=-0.5)
    # broadcast group stats back to channels, per-channel A/B, xn = x*A + B
    for i in range(CT):
        mr_ps = PS.tile([128, 2], F32, name="mr_ps", tag="sc")
        nc.tensor.matmul(out=mr_ps, lhsT=gselt_t[:, i * 128:(i + 1) * 128],
                         rhs=mvr_sb, start=True, stop=True)
        abm = W.tile([128, 1], F32, name="abm", tag="abm", bufs=4)
        nc.vector.tensor_mul(out=ab_t[i][:, 0:1], in0=mr_ps[:, 1:2], in1=gamma_sb[:, i:i + 1])
        nc.vector.tensor_mul(out=abm, in0=mr_ps[:, 0:1], in1=ab_t[i][:, 0:1])
        nc.vector.tensor_sub(out=ab_t[i][:, 1:2], in0=beta_sb[:, i:i + 1], in1=abm)
        nc.vector.tensor_scalar(out=xn_t[i], in0=x_t[i],
                                scalar1=ab_t[i][:, 0:1], scalar2=ab_t[i][:, 1:2],
                                op0=mybir.AluOpType.mult, op1=mybir.AluOpType.add)

    # ---- deferred PE work: 4-matmul units spread across pairs 0..2 ----
    def qk_unit(oc, t2):
        p, is_k = oc // 2, oc % 2
        fs = slice(t2 * 512, t2 * 512 + 512)
        ps = PS.tile([128, 512], F32, name="qkps", tag="sc")
        for kk in range(CT):
            nc.tensor.matmul(out=ps,
                             lhsT=wqk_t[:, kk, oc * 128:(oc + 1) * 128],
                             rhs=xn_t[kk][:, fs],
                             start=(kk == 0), stop=(kk == CT - 1))
        dst = (k_t[p] if is_k else q_t[p])[:, fs]
        if use_bqk:
            nc.vector.tensor_scalar(out=dst, in0=ps, scalar1=bqk_sb[:, oc:oc + 1],
                                    scalar2=None, op0=mybir.AluOpType.add)
        else:
            nc.vector.tensor_copy(out=dst, in_=ps)

    def vt_unit(s):
        # whole-tile memset (strided ones-column memset fails this walrus);
        # the copy below overwrites cols 0..63 per head, col 64 stays 1.0
        nc.vector.memset(vaug_t[s], 1.0)
        ps = PS.tile([128, C], F32, name="vtps", tag="sc")
        for kk in range(CT):
            nc.tensor.matmul(out=ps,
                             lhsT=xn_t[kk][:, s * 128:(s + 1) * 128],
                             rhs=wv_t[:, kk, :],
                             start=(kk == 0), stop=(kk == CT - 1))
        nc.vector.tensor_copy(out=vaug_t[s][:, :, 0:CH],
                              in_=ps.rearrange("p (h e) -> p h e", e=CH))

    for oc in range(2):
        for t2 in range(TC):
            qk_unit(oc, t2)
    # (pair, ss) -> work units; vT for s must land by the position where
    # wv consumes vaug[s] (DELAY steps after scores s, counting the spill
    # into the next pair); qk for pair p anywhere before pair p
    sched = {}
    for s in range(6):
        sched.setdefault((0, s), []).append(lambda s=s: vt_unit(s))
    sched.setdefault((1, 0), []).append(lambda: vt_unit(6))
    sched.setdefault((1, 1), []).append(lambda: vt_unit(7))
    for j, (oc, t2) in enumerate([(oc, t2) for oc in (2, 3) for t2 in range(TC)]):
        sched.setdefault((0, 6 + j // 2), []).append(lambda oc=oc, t2=t2: qk_unit(oc, t2))
    for j, (oc, t2) in enumerate([(oc, t2) for oc in (4, 5) for t2 in range(TC)]):
        sched.setdefault((1, 2 + j), []).append(lambda oc=oc, t2=t2: qk_unit(oc, t2))
    for j, (oc, t2) in enumerate([(oc, t2) for oc in (6, 7) for t2 in range(TC)]):
        sched.setdefault((2, j), []).append(lambda oc=oc, t2=t2: qk_unit(oc, t2))

    # ---- attention: head pairs, wv DELAY steps behind scores/exp ----
    def division(p, hu_ps, htmp):
        """h = hu/D for pair p, into h_t[p].

        1/D on DVE costs 8 cycles/element of FREE dim, so the four D rows
        (4 x 512) are DMA-reshaped to (64,32) first: one 0.4us reciprocal.
        The reciprocal then takes a DRAM round trip so a partition-broadcast
        DMA (stride-0 partition reads are DRAM-only) can replicate each
        512-vector across 64 partitions for the final psum*sbuf multiply.
        Nothing lands on the scalar engine (pacing exp) or PE.
        """
        chains = [(half, t2) for half in range(2) for t2 in range(TC)]
        scr = DR.tile([4, 512], F32, name="scr", tag="scr", bufs=2)
        scr2 = DR.tile([4, 512], F32, name="scr2", tag="scr2", bufs=2)
        dall = W.tile([64, 32], F32, name="dall", tag="dall", bufs=2)
        ralt = W.tile([64, 32], F32, name="ralt", tag="ralt", bufs=2)
        husb = []
        for j, (half, t2) in enumerate(chains):
            hs = W.tile([128, 512], F32, name="husb", tag="husb", bufs=8)
            husb.append(hs)
            # copy h rows + D row out of PSUM so the accumulate banks free
            # immediately instead of waiting out the division's DMA latency
            nc.vector.tensor_copy(out=hs[0:CH + 1, :],
                                  in_=hu_ps[half][t2][0:CH + 1, :])
            nc.sync.dma_start(out=scr[j:j + 1, :], in_=hs[CH:CH + 1, :])
        nc.sync.dma_start(out=dall,
                          in_=scr.rearrange("a (b c) -> (a b) c", c=32))
        nc.vector.reciprocal(out=ralt, in_=dall)
        nc.sync.dma_start(out=scr2.rearrange("a (b c) -> (a b) c", c=32),
                          in_=ralt)
        for j, (half, t2) in enumerate(chains):
            fs = slice(t2 * 512, t2 * 512 + 512)
            rcb = W.tile([64, 512], F32, name="rcb", tag="rcb", bufs=4)
            row = scr2[j:j + 1, :]
            bcast = bass.AP(tensor=row.tensor, offset=row.offset,
                            ap=[[0, 64], list(row.ap[-1])])
            nc.sync.dma_start(out=rcb, in_=bcast)
            dst = h_t[p] if half == 0 else htmp
            with nc.allow_low_precision(reason="f32r tag on h; same payload"):
                nc.vector.tensor_mul(out=dst[0:CH, fs],
                                     in0=husb[j][0:CH, :],
                                     in1=rcb)
        # odd head's h goes to partitions 64..127 (cross-partition -> DMA)
        nc.sync.dma_start(out=h_t[p][64:128, :], in_=htmp[0:64, :])

    def division_act(p, hu_ps, htmp):
        """Tail variant: 1/D = exp(-ln D) on the (by now idle) scalar engine
        after a K=1 ones-matmul broadcasts D across partitions - avoids the
        ~10us of DMA round-trip latency the DMA-based division would expose
        at the end of the kernel."""
        for half in range(2):
            for t2 in range(TC):
                fs = slice(t2 * 512, t2 * 512 + 512)
                dsb = W.tile([128, 512], F32R, name="dsbt", tag="dsb", bufs=4)
                nc.vector.tensor_copy(out=dsb[CH:CH + 1, :],
                                      in_=hu_ps[half][t2][CH:CH + 1, :])
                bc = PS.tile([128, 512], F32, name="bcps", tag="sc")
                nc.tensor.matmul(out=bc, lhsT=ones_t[CH:CH + 1, :],
                                 rhs=dsb[CH:CH + 1, :], start=True, stop=True)
                lnd = W.tile([128, 512], F32, name="lnd", tag="lnd", bufs=2)
                nc.scalar.activation(out=lnd, in_=bc,
                                     func=mybir.ActivationFunctionType.Ln)
                rc = W.tile([128, 512], F32, name="rct", tag="rct", bufs=2)
                nc.scalar.activation(out=rc, in_=lnd,
                                     func=mybir.ActivationFunctionType.Exp,
                                     scale=-1.0)
                dst = h_t[p] if half == 0 else htmp
                with nc.allow_low_precision(reason="f32r tag on h; same payload"):
                    nc.vector.tensor_mul(out=dst[0:CH, fs],
                                         in0=hu_ps[half][t2][0:CH, :],
                                         in1=rc[0:CH, :])
        nc.sync.dma_start(out=h_t[p][64:128, :], in_=htmp[0:64, :])

    pend = None  # (p, hu_ps, htmp) awaiting division
    for p in range(4):
        hu_ps = [[None] * TC for _ in range(2)]
        htmp = W.tile([64, T], BF16, name="htmp", tag="htmp", bufs=2)
        expw = {}
        prev = pend
        for ss in range(ST):
            scps = [None, None]
            for half in range(2):
                base = 64 * half
                sc = PS.tile([128, T], F32, name=f"scps{half}", tag="sc")
                scps[half] = sc
                for t2 in range(TC):
                    fs = slice(t2 * 512, t2 * 512 + 512)
                    nc.tensor.matmul(
                        out=sc[:, fs],
                        lhsT=k_t[p][base:base + 64, ss * 128:(ss + 1) * 128],
                        rhs=q_t[p][base:base + 64, fs],
                        start=True, stop=True)
                ew = W.tile([128, T], BF16, name="expw", tag="expw", bufs=14)
                nc.scalar.activation(out=ew, in_=scps[half],
                                     func=mybir.ActivationFunctionType.Exp,
                                     scale=0.125)
                expw[half, ss] = ew
            for unit in sched.get((p, ss), []):
                unit()
            if prev is not None:
                # previous pair's wv tail (ss ST-DELAY..ST-1), then division
                if ss < DELAY:
                    _wv(nc, PS, vaug_t, prev[3], prev[1], prev[0], ST - DELAY + ss)
                if ss == DELAY - 1:
                    division(prev[0], prev[1], prev[2])
                    prev = pend = None
            if ss >= DELAY:
                _wv(nc, PS, vaug_t, expw, hu_ps, p, ss - DELAY)
        pend = (p, hu_ps, htmp, expw)
    for ss in range(ST - DELAY, ST):
        _wv(nc, PS, vaug_t, pend[3], pend[1], pend[0], ss)
    division_act(pend[0], pend[1], pend[2])

    # ---- proj + residual ----
    for oc in range(CT):
        ps = PS.tile([128, T], F32, name="prps", tag="sc")
        for t2 in range(TC):
            fs = slice(t2 * 512, t2 * 512 + 512)
            for kk in range(CT):
                nc.tensor.matmul(out=ps[:, fs],
                                 lhsT=wp_t[:, kk, oc * 128:(oc + 1) * 128],
                                 rhs=h_t[kk][:, fs],
                                 start=(kk == 0), stop=(kk == CT - 1))
        if use_bproj:
            prtmp = W.tile([128, T], F32, name="prtmp", tag="prtmp", bufs=2)
            nc.vector.tensor_scalar(out=prtmp, in0=ps, scalar1=bproj_sb[:, oc:oc + 1],
                                    scalar2=None, op0=mybir.AluOpType.add)
            with nc.allow_low_precision(reason="f32r tag on out; same payload"):
                nc.vector.tensor_add(out=x_t[oc], in0=x_t[oc], in1=prtmp)
        else:
            with nc.allow_low_precision(reason="f32r tag on out; same payload"):
                nc.vector.tensor_add(out=x_t[oc], in0=x_t[oc], in1=ps)
        for t2 in range(TC):
            fs = slice(t2 * 512, t2 * 512 + 512)
            nc.sync.dma_start(out=out_d[oc * 128:(oc + 1) * 128, fs],
                              in_=x_t[oc][:, fs])


def _wv(nc, PS, vaug_t, expw, hu_ps, p, ss):
    for half in range(2):
        h = 2 * p + half
        for t2 in range(TC):
            if ss == 0:
                hu_ps[half][t2] = PS.tile([128, 512], F32,
                                          name="hups", tag="hu", bufs=4)
            fs = slice(t2 * 512, t2 * 512 + 512)
            nc.tensor.matmul(out=hu_ps[half][t2][0:CH + 1, :],
                             lhsT=vaug_t[ss][:, h, :],
                             rhs=expw[half, ss][:, fs],
                             start=(ss == 0), stop=(ss == ST - 1))


_PROGRAM_CACHE = {}


def _get_program(use_bqk, use_bproj):
    key = (use_bqk, use_bproj)
    if key not in _PROGRAM_CACHE:
        _PROGRAM_CACHE[key] = build_program(*key)
    return _PROGRAM_CACHE[key]


def make_host_inputs(x, gamma, beta, w_qkv, b_qkv, w_proj, b_proj):
    """Host-side preprocessing shared by all cores."""
    x = np.asarray(x, np.float32)
    w_qkv = np.asarray(w_qkv, np.float32)
    b_qkv = np.asarray(b_qkv, np.float32)
    w_proj = np.asarray(w_proj, np.float32)
    b_proj = np.asarray(b_proj, np.float32)
    gamma = np.asarray(gamma, np.float32)
    beta = np.asarray(beta, np.float32)

    # per-head slices of w_qkv rows (3c, c): head h -> q,k,v at 192h+{0,64,128}
    wq = np.stack([w_qkv[192 * h:192 * h + 64] for h in range(NH)])
    wk = np.stack([w_qkv[192 * h + 64:192 * h + 128] for h in range(NH)])
    wv = np.stack([w_qkv[192 * h + 128:192 * h + 192] for h in range(NH)])
    bq = np.stack([b_qkv[192 * h:192 * h + 64] for h in range(NH)])
    bk = np.stack([b_qkv[192 * h + 64:192 * h + 128] for h in range(NH)])
    bv = np.stack([b_qkv[192 * h + 128:192 * h + 192] for h in range(NH)])

    # wqk (512c, 1024): chunk 2p = q of heads (2p,2p+1), chunk 2p+1 = k of same
    chunks, bqk_chunks = [], []
    for p in range(4):
        chunks.append(np.concatenate([wq[2 * p], wq[2 * p + 1]], 0).T)
        chunks.append(np.concatenate([wk[2 * p], wk[2 * p + 1]], 0).T)
        bqk_chunks.append(np.concatenate([bq[2 * p], bq[2 * p + 1]], 0))
        bqk_chunks.append(np.concatenate([bk[2 * p], bk[2 * p + 1]], 0))
    wqk_host = np.concatenate(chunks, axis=1)                     # (512,1024)
    bqk_host = np.stack(bqk_chunks, axis=1)                       # (128,8)

    wvT_host = wv.reshape(C, C).T.copy()                          # (512c, 512vch)
    wprojT_host = w_proj.T.copy()                                 # (512c, 512o)
    # v-bias contributes exactly b_v through the softmax (rows sum to 1);
    # fold it into the proj bias
    bproj_eff = b_proj + w_proj @ bv.reshape(C)
    bproj_host = bproj_eff.reshape(CT, 128).T.copy()
    gamma_host = gamma.reshape(CT, 128).T.copy()
    beta_host = beta.reshape(CT, 128).T.copy()

    cidx = np.arange(C)
    # gsel aggregates raw [sum, sum(x^2)] rows -> per-group means
    gsel_host = (cidx[:, None] // GS == np.arange(NG)[None, :]).astype(np.float32) / (GS * T)
    gselt_host = (cidx[None, :] // GS == np.arange(NG)[:, None]).astype(np.float32)

    def ktile(a):
        # (512, N) -> (128, 4, N): partition-major layout for one big tile
        return np.ascontiguousarray(a.reshape(CT, 128, -1).transpose(1, 0, 2))

    use_bqk = bool(np.any(bqk_host))
    use_bproj = bool(np.any(bproj_host))
    biases = np.concatenate([bqk_host, bproj_host, gamma_host, beta_host], axis=1)

    import ml_dtypes
    common = {
        "wqk": ktile(wqk_host).astype(ml_dtypes.bfloat16),
        "wv": ktile(wvT_host).astype(ml_dtypes.bfloat16),
        "wproj": ktile(wprojT_host).astype(ml_dtypes.bfloat16),
        "gsel": ktile(gsel_host),
        "gselt": np.ascontiguousarray(gselt_host),
        "biases": np.ascontiguousarray(biases.astype(np.float32)),
    }
    return x, common, use_bqk, use_bproj


def kernel(x, gamma, beta, w_qkv, b_qkv, w_proj, b_proj):
    b, c, H, Wd = x.shape
    assert (b, c, H, Wd) == (8, C, 32, 32)
    xf, common, use_bqk, use_bproj = make_host_inputs(
        x, gamma, beta, w_qkv, b_qkv, w_proj, b_proj)
    xf = xf.reshape(b, C, T)

    nc = _get_program(use_bqk, use_bproj)
    if not getattr(nc, "_waits_split", False):
        _split_multi_waits(nc)
        nc._waits_split = True
    in_maps = [dict(common, x=np.ascontiguousarray(xf[i])) for i in range(NCORES)]
    res = run_bass_kernel_spmd(nc, in_maps, list(range(NCORES)))
    out = np.stack([res.results[i]["out"] for i in range(NCORES)])
    return out.reshape(b, C, H, Wd).astype(np.float32)


if __name__ == "__main__":
    rng = np.random.default_rng(0)
    args = {
        "x": rng.standard_normal((8, C, 32, 32), dtype=np.float32),
        "gamma": np.ones(C, np.float32),
        "beta": np.zeros(C, np.float32),
        "w_qkv": (rng.standard_normal((3 * C, C)) * 0.02).astype(np.float32),
        "b_qkv": np.zeros(3 * C, np.float32),
        "w_proj": (rng.standard_normal((C, C)) * 0.02).astype(np.float32),
        "b_proj": np.zeros(C, np.float32),
    }
    out = kernel(**args)
    print(out.shape, out.dtype)
